# revision 1
# baseline (speedup 1.0000x reference)
"""Trainium2 Bass kernel for the batched MPS quantum-circuit forward pass.

Math: every gate update in the reference circuit is local to one site, and the
CNOT MPO application is pure index bookkeeping (A_CTRL/B_TGT are 0/1 tensors).
Writing lam = (m0 m1 m2 m3) for the left-bond bits and rho = (a0 a1 a2 a3) for
the right-bond bits, the final site tensor factorizes in closed form:

  interior q:  T[q][lam, rho, i] = delta(i, a3) * prod_l U_l[a_l ^ m_l, a_{l-1}]
  site 0:      same with m = 0 (only lam = 0 nonzero)
  site 19:     T[19][lam, 0, i]  = sum_{a0 a1 a2} (same product), i = a3

with U_l = RZ(z_l) RY(y_l) the per-(batch, qubit, layer) 2x2 gate and
a_{-1} = 0.  So the kernel computes the four gate entry tables, the pairwise
chain products C01 = F0*F1 (16/site) and C23 = F2*F3 (32/site), then expands
out[lam, rho] = C01[m0 m1 a0 a1] * C23[m2 m3 a1 a2 a3] with gather-style
access patterns, writing straight into the interleaved complex64 layout.

Sharding: pure data parallelism - batch 1024 is split 128 per core across the
8 cores (partition dim = batch).
"""

import sys

sys.path.insert(0, "/opt/trn_rl_repo")

import numpy as np

B_TOTAL = 1024
N_CORES = 8
B = B_TOTAL // N_CORES  # 128 rows per core == SBUF partitions
NQ = 20
P_COLS = 160
ROW_F32 = NQ * 16 * 16 * 2 * 2  # 20480 fp32 per batch row (interleaved complex)

_CACHE = {}


def _build_nc():
    import concourse.bass as bass
    import concourse.tile as tile
    from concourse import bacc, mybir

    f32 = mybir.dt.float32
    MUL = mybir.AluOpType.mult
    SIN = mybir.ActivationFunctionType.Sin

    nc = bacc.Bacc("TRN2", target_bir_lowering=False, debug=False)
    theta_d = nc.dram_tensor("theta", [B, P_COLS], f32, kind="ExternalInput").ap()
    out_d = nc.dram_tensor("out", [B, ROW_F32], f32, kind="ExternalOutput").ap()

    from contextlib import ExitStack

    with tile.TileContext(nc) as tc, ExitStack() as ctx:
        pool = ctx.enter_context(tc.tile_pool(name="main", bufs=1))

        def tl(name, w):
            return pool.tile([B, w], f32, name=name)

        th = tl("th", 160)
        sinv = tl("sinv", 160)
        cosv = tl("cosv", 160)
        halfpi = tl("halfpi", 1)
        p8 = tl("p8", 640)          # zones of 80: cc sc cs ss -cc -sc -cs -ss
        f0 = tl("f0", 160)          # [m0,a0,q] re | im
        f1 = tl("f1", 320)          # [m1,a1,a0,q] re | im
        f2 = tl("f2", 320)          # [m2,a2,a1,q]
        f3 = tl("f3", 320)          # [m3,a3,a2,q]
        c01 = tl("c01", 640)        # per site 16: m0*8+m1*4+a0*2+a1 ; re | im
        c23 = tl("c23", 1280)       # per site 32: m2*16+m3*8+a1*4+a2*2+a3 ; re | im
        ca = tl("ca", 320)
        cb = tl("cb", 320)
        cc_s = tl("cc_s", 640)
        cd_s = tl("cd_s", 640)
        t1 = tl("t1", 512)
        t2 = tl("t2", 512)
        t3 = tl("t3", 512)
        t4 = tl("t4", 512)
        tp1 = tl("tp1", 512)
        tp2 = tl("tp2", 512)
        tp3 = tl("tp3", 512)
        tp4 = tl("tp4", 512)
        s01 = tl("s01", 16)
        s02 = tl("s02", 16)
        s03 = tl("s03", 16)
        s04 = tl("s04", 16)
        u19a = tl("u19a", 256)
        u19b = tl("u19b", 256)
        pr19 = tl("pr19", 256)
        pi19 = tl("pi19", 256)
        r1r = tl("r1r", 128)
        r1i = tl("r1i", 128)
        r2r = tl("r2r", 64)
        r2i = tl("r2i", 64)
        sr = tl("sr", 32)
        si = tl("si", 32)
        outa = tl("outa", 7 * 1024)   # sites 0..6
        outb = tl("outb", 6 * 1024)   # sites 7..12
        outc = tl("outc", 6 * 1024)   # sites 13..18
        outd = tl("outd", 1024)       # site 19

        def ap(t, off, dims):
            w = t.shape[1]
            return bass.AP(tensor=t.tensor, offset=t.offset + off, ap=[[w, B]] + dims)

        # ---- stage A: angles -> sin/cos of half-angles --------------------
        nc.vector.memset(halfpi[:], float(np.pi / 2))
        warm = tl("warm", 1)
        nc.scalar.activation(warm[:], halfpi[:], SIN, scale=0.5)
        nc.sync.dma_start(th[:], theta_d)
        nc.scalar.activation(sinv[:], th[:], SIN, scale=0.5)
        # cos(x) = sin(pi/2 - |x|), keeps the Sin operand inside [-pi, pi]
        absv = tl("absv", 160)
        nc.scalar.activation(absv[:], th[:], mybir.ActivationFunctionType.Abs, scale=0.5)
        nc.scalar.activation(cosv[:], absv[:], SIN, bias=halfpi[:], scale=-1.0)

        # ---- stage B: base products p8 -----------------------------------
        # theta col = l*40 + g*20 + q ; g=0 -> RY(y), g=1 -> RZ(z)
        # zone z col = z*80 + l*20 + q
        # cc = cos(y/2)cos(z/2), sc = cos(y/2)sin(z/2),
        # cs = sin(y/2)cos(z/2), ss = sin(y/2)sin(z/2)
        lq = [[20, 4], [1, 20]]
        thlq = [[40, 4], [1, 20]]
        for zone, (g0, g1) in enumerate([(cosv, cosv), (cosv, sinv), (sinv, cosv), (sinv, sinv)]):
            nc.vector.tensor_tensor(
                ap(p8, zone * 80, lq), ap(g0, 0, thlq), ap(g1, 20, thlq), MUL
            )
        nc.vector.tensor_scalar_mul(ap(p8, 320, [[1, 320]]), ap(p8, 0, [[1, 320]]), -1.0)

        # ---- stage C: gate-entry tables F0..F3 ---------------------------
        # U[r,c]: re zone: r==c -> cc(0); (0,1) -> -cs(6); (1,0) -> cs(2)
        #         im zone: (0,0) -> -sc(5); (1,1) -> sc(1); r!=c -> ss(3)
        Z = {"cc": 0, "sc": 80, "cs": 160, "ss": 240, "-cc": 320, "-sc": 400, "-cs": 480, "-ss": 560}
        # F1..F3: idx mb*80 + ab*40 + cb*20 + q, plane im at +160, zone col +20*l
        for ftab, l in ((f2, 2), (f3, 3), (f1, 1)):
            off = 20 * l
            # group A: (mb,ab) in {(0,0),(1,1)} -> row 0; out bases 0,120
            # group B: {(0,1),(1,0)} -> row 1; out bases 40,80
            for plane, zr0, zr1 in (
                (0, (Z["cc"], Z["-cs"]), (Z["cs"], Z["cc"])),      # re: row0: c=0 cc, c=1 -cs ; row1: cs, cc
                (160, (Z["-sc"], Z["ss"]), (Z["ss"], Z["sc"])),    # im
            ):
                nc.scalar.copy(
                    ap(ftab, plane + 0, [[120, 2], [20, 2], [1, 20]]),
                    ap(p8, zr0[0] + off, [[0, 2], [zr0[1] - zr0[0], 2], [1, 20]]),
                )
                nc.scalar.copy(
                    ap(ftab, plane + 40, [[40, 2], [20, 2], [1, 20]]),
                    ap(p8, zr1[0] + off, [[0, 2], [zr1[1] - zr1[0], 2], [1, 20]]),
                )

        # ---- stage D: C01 = F0 * F1 --------------------------------------
        # traversal (q, m1, a0, a1), split by m0; C01 site stride 16
        # F0[m0,a0] = U0[a0^m0, 0]: read directly from p8 (l=0) - no f0 table
        F0B = {(0, 0): (0, 160), (0, 1): (160, -160),
               (80, 0): (400, -160), (80, 1): (240, 160)}

        def c01_mult(dst, f0_plane, f1_plane):
            for m0 in (0, 1):
                b0, s0 = F0B[(f0_plane, m0)]
                for m1 in (0, 1):
                    nc.vector.tensor_tensor(
                        ap(dst, m0 * 8 + m1 * 4, [[16, 20], [2, 2], [1, 2]]),
                        ap(p8, b0, [[1, 20], [s0, 2], [0, 2]]),
                        ap(f1, f1_plane + m1 * 80, [[1, 20], [20, 2], [40, 2]]),
                        MUL,
                    )

        c01_mult(ca, 0, 0)      # rr
        c01_mult(cb, 80, 160)   # ii
        nc.vector.tensor_sub(ap(c01, 0, [[1, 320]]), ca[:], cb[:])
        c01_mult(ca, 0, 160)    # ri
        c01_mult(cb, 80, 0)     # ir
        nc.vector.tensor_add(ap(c01, 320, [[1, 320]]), ca[:], cb[:])

        # ---- stage E: C23 = F2 * F3 --------------------------------------
        # traversal (q, m3, a1, a2), split by (m2, a3); C23 site stride 32
        def c23_mult(dst, f2_plane, f3_plane):
            for m2 in (0, 1):
                for m3 in (0, 1):
                    for a3 in (0, 1):
                        eng23 = nc.gpsimd if (m3 + a3) % 2 else nc.vector
                        eng23.tensor_tensor(
                            ap(dst, m2 * 16 + m3 * 8 + a3, [[32, 20], [4, 2], [2, 2]]),
                            ap(f2, f2_plane + m2 * 80, [[1, 20], [20, 2], [40, 2]]),
                            ap(f3, f3_plane + m3 * 80 + a3 * 40, [[1, 20], [0, 2], [20, 2]]),
                            MUL,
                        )

        c23_mult(cc_s, 0, 0)
        c23_mult(cd_s, 160, 160)
        nc.vector.tensor_sub(ap(c23, 0, [[1, 640]]), cc_s[:], cd_s[:])
        c23_mult(cc_s, 0, 160)
        c23_mult(cd_s, 160, 0)
        nc.vector.tensor_add(ap(c23, 640, [[1, 640]]), cc_s[:], cd_s[:])

        # ---- hole memsets (positions that stay zero) ---------------------
        # interior sites: per rho-highpair block of 8 fp32, holes at +2..+5
        for outt, qrel, nsites in ((outa, 1, 6), (outb, 0, 6), (outc, 0, 6)):
            nc.gpsimd.memset(
                ap(outt, qrel * 1024 + 2, [[1024, nsites], [8, 128], [1, 4]]), 0.0
            )
        nc.gpsimd.memset(ap(outa, 64, [[1, 960]]), 0.0)            # site 0, lam > 0
        nc.gpsimd.memset(ap(outa, 2, [[8, 8], [1, 4]]), 0.0)       # site 0 holes in lam=0 row
        nc.gpsimd.memset(ap(outd, 4, [[64, 16], [1, 60]]), 0.0)    # site 19, rho > 0

        # ---- stage G: site 0 (m = 0 chain only) --------------------------
        for a1 in (0, 1):
            sdim = [[8, 2], [1, 4]]  # (a0, a2a3) scratch slice at a1*4
            A0 = lambda pl: ap(c01, pl + a1, [[2, 2], [0, 4]])
            B0 = lambda pl: ap(c23, pl + a1 * 4, [[0, 2], [1, 4]])
            nc.vector.tensor_tensor(ap(s01, a1 * 4, sdim), A0(0), B0(0), MUL)
            nc.vector.tensor_tensor(ap(s02, a1 * 4, sdim), A0(320), B0(640), MUL)
            nc.vector.tensor_tensor(ap(s03, a1 * 4, sdim), A0(0), B0(640), MUL)
            nc.vector.tensor_tensor(ap(s04, a1 * 4, sdim), A0(320), B0(0), MUL)
            o0 = [[32, 2], [8, 2], [6, 2]]
            sd2 = [[8, 2], [2, 2], [1, 2]]
            nc.vector.tensor_sub(
                ap(outa, a1 * 16, o0), ap(s01, a1 * 4, sd2), ap(s02, a1 * 4, sd2)
            )
            nc.vector.tensor_add(
                ap(outa, a1 * 16 + 1, o0), ap(s03, a1 * 4, sd2), ap(s04, a1 * 4, sd2)
            )

        def _emit_site19():
            # ---- stage H: site 19 (sum over a0,a1,a2; rho = 0) ---------------
            # scratch layout: a0*256 + a3*128 + lamA*32 + lamB*8 + a1*4? no:
            # (lamA,lamB,a1,a2) -> strides 16,4,2,1 within 64-block
            def p19_mult(dst, c01_pl, c23_pl):
                for a0 in (0, 1):
                    for a3 in (0, 1):
                        for a1 in (0, 1):
                            nc.vector.tensor_tensor(
                                ap(dst, a0 * 128 + a3 * 64 + a1 * 2, [[16, 4], [4, 4], [1, 2]]),
                                ap(c01, c01_pl + 19 * 16 + a0 * 2 + a1, [[4, 4], [0, 4], [0, 2]]),
                                ap(c23, c23_pl + 19 * 32 + a1 * 4 + a3, [[0, 4], [8, 4], [2, 2]]),
                                MUL,
                            )

            p19_mult(u19a, 0, 0)
            p19_mult(u19b, 320, 640)
            nc.vector.tensor_sub(pr19[:], u19a[:], u19b[:])
            p19_mult(u19a, 0, 640)
            p19_mult(u19b, 320, 0)
            nc.vector.tensor_add(pi19[:], u19a[:], u19b[:])
            # reduce a0 (stride 256), then a1 (stride 2), then a2 (stride 1)
            for src, d1, d2, dst in ((pr19, r1r, r2r, sr), (pi19, r1i, r2i, si)):
                nc.vector.tensor_add(d1[:], src[:, 0:128], src[:, 128:256])
                nc.vector.tensor_add(
                    ap(d2, 0, [[32, 2], [2, 16], [1, 2]]),
                    ap(d1, 0, [[64, 2], [4, 16], [1, 2]]),
                    ap(d1, 2, [[64, 2], [4, 16], [1, 2]]),
                )
                nc.vector.tensor_add(
                    ap(dst, 0, [[16, 2], [1, 16]]),
                    ap(d2, 0, [[32, 2], [2, 16]]),
                    ap(d2, 1, [[32, 2], [2, 16]]),
                )
            # scatter: out[19][lam, 0, i=a3] at lam*64 + a3*2 (+1 im)
            nc.scalar.copy(
                ap(outd, 0, [[2, 2], [64, 16]]), ap(sr, 0, [[16, 2], [1, 16]])
            )
            nc.scalar.copy(
                ap(outd, 1, [[2, 2], [64, 16]]), ap(si, 0, [[16, 2], [1, 16]])
            )
            nc.sync.dma_start(out_d[:, 19 * 1024 : 20 * 1024], outd[:])
        import os
        PN = [int(x) for x in os.environ.get("KERN_POOL_NS", "3,3,3").split(",")]
        for gi, (outt, qb, qrel, nsq) in enumerate(
            ((outa, 1, 1, 6), (outb, 7, 0, 6), (outc, 13, 0, 6))
        ):
            pool_n = PN[gi]
            for a1 in (0, 1):
                for a2 in (0, 1):
                    for a3 in (0, 1):
                        trip = a1 * 4 + a2 * 2 + a3
                        scr = [[16, nsq], [4, 4], [1, 4]]
                        if trip >= 8 - pool_n:
                            eng, w1, w2, w3, w4 = nc.gpsimd, tp1, tp2, tp3, tp4
                        else:
                            eng, w1, w2, w3, w4 = nc.vector, t1, t2, t3, t4
                        for a0 in (0, 1):
                            A = lambda pl: ap(
                                c01, pl + qb * 16 + a0 * 2 + a1, [[16, nsq], [4, 4], [0, 4]]
                            )
                            Bv = lambda pl: ap(
                                c23,
                                pl + qb * 32 + a1 * 4 + a2 * 2 + a3,
                                [[32, nsq], [0, 4], [8, 4]],
                            )
                            h = (a0 * 2 + a1) * 128
                            eng.tensor_tensor(ap(w1, h, scr), A(0), Bv(0), MUL)
                            eng.tensor_tensor(ap(w2, h, scr), A(320), Bv(640), MUL)
                            eng.tensor_tensor(ap(w3, h, scr), A(0), Bv(640), MUL)
                            eng.tensor_tensor(ap(w4, h, scr), A(320), Bv(0), MUL)
                        ob = qrel * 1024 + a1 * 16 + a2 * 8 + a3 * 6
                        odims = [[1024, nsq], [64, 16], [32, 2]]
                        sdims = [[16, nsq], [1, 16], [256, 2]]
                        hh = a1 * 128
                        eng.tensor_sub(
                            ap(outt, ob, odims), ap(w1, hh, sdims), ap(w2, hh, sdims)
                        )
                        eng.tensor_add(
                            ap(outt, ob + 1, odims), ap(w3, hh, sdims), ap(w4, hh, sdims)
                        )
            if outt is outa:
                nc.sync.dma_start(out_d[:, 0 : 7 * 1024], outa[:])
            else:
                base = (qb - qrel) * 1024
                nc.sync.dma_start(out_d[:, base : base + nsq * 1024], outt[:])

        _emit_site19()

        # ---- stage F: wide expansion, interior sites ---------------------
        # out fp32 offset within site block: lamA*256 + lamB*64 + a0*32 + a1*16
        #                                    + a2*8 + a3*6 (+1 for im)

    nc.compile()
    return nc


def _get_nc():
    if "nc" not in _CACHE:
        _CACHE["nc"] = _build_nc()
    return _CACHE["nc"]


def kernel(theta, batch_size):
    from concourse.bass_utils import run_bass_kernel_spmd

    theta = np.ascontiguousarray(np.asarray(theta), dtype=np.float32)
    assert theta.shape == (B_TOTAL, P_COLS)
    nc = _get_nc()
    in_maps = [
        {"theta": theta[c * B : (c + 1) * B]} for c in range(N_CORES)
    ]
    res = run_bass_kernel_spmd(nc, in_maps, core_ids=list(range(N_CORES)))
    _CACHE["last_res"] = res
    full = np.concatenate([r["out"] for r in res.results], axis=0)  # [1024, 20480] f32
    return full.view(np.complex64).reshape(B_TOTAL, NQ, 16, 16, 2)



# revision 6
# speedup vs baseline: 1.2489x; 1.2489x over previous
"""Trainium2 Bass kernel for the batched MPS quantum-circuit forward pass (v6).

Math identical to v3-v5 (Gauss 3-mult complex products, fp16 intermediates,
q-innermost layouts). v6 restructures every op so each OPERAND has at most
3 free dims after AP optimization (hardware TENSOR3D codegen limit), using
index orders chosen so contiguity merges collapse the emitted dims:

  sc2:  cos@0, sin@160; col = l*40 + g*20 + q
  p8:   zone*80 + l*20 + q; zones cc sc cs ss / -sc@400 -cs@480
  f123: l'*480 + plane*160 + m*80 + a*40 + c*20 + q (planes re, im, nim)
  cab:  t*640 + k*320 + idx01*20 + q
  c01:  plane*320 + idx01*20 + q, idx01 = a1*8 + a0*4 + m0*2 + m1
        (planes re, im, S = re+im)
  ccd:  kpair*1280 + k*640 + idx23*20 + q
  c23:  plane*640 + idx23*20 + q, idx23 = m2*16 + m3*8 + a3*4 + a2*2 + a1
        (planes P2' = re-im, P3 = re+im, re, imt)
  T:    k*256ns + a1*128ns + a0*64ns + m01*16ns + m23*4ns + a3*2ns + a2*ns + s
  t0s:  k*16 + a0*8 + a1*4 + a2*2 + a3
  t19:  k*256 + a1*128 + a0*64 + m01*16 + m23*4 + a3*2 + a2
"""

import sys

sys.path.insert(0, "/opt/trn_rl_repo")

import numpy as np

B_TOTAL = 1024
N_CORES = 8
B = B_TOTAL // N_CORES
NQ = 20
P_COLS = 160
ROW_F32 = NQ * 16 * 16 * 2 * 2

_CACHE = {}

CHUNKS = [(1, 2), (3, 2), (5, 3), (8, 4), (12, 4), (16, 3)]
# engine for chunk sub-ops, one char per (a1,a3) quarter: v=DVE g=Pool
SUBENG = ["vvvv", "vvvv", "vvvv", "vvgg", "vvgg", "vvgg"]


def _build_nc():
    import concourse.bass as bass
    import concourse.tile as tile
    from concourse import bacc, mybir

    f32 = mybir.dt.float32
    f16 = mybir.dt.float16
    MUL = mybir.AluOpType.mult
    SIN = mybir.ActivationFunctionType.Sin

    nc = bacc.Bacc("TRN2", target_bir_lowering=False, debug=False)
    theta_d = nc.dram_tensor("theta", [B, P_COLS], f32, kind="ExternalInput").ap()
    out_d = nc.dram_tensor("out", [B, ROW_F32], f32, kind="ExternalOutput").ap()

    from contextlib import ExitStack

    with tile.TileContext(nc) as tc, ExitStack() as ctx:
        pool = ctx.enter_context(tc.tile_pool(name="main", bufs=1))

        def tl(name, w, dt=f16):
            return pool.tile([B, w], dt, name=name)

        th = tl("th", 160, f32)
        absv = tl("absv", 160, f32)
        negh = tl("negh", 160, f32)
        halfpi = tl("halfpi", 1, f32)
        warm = tl("warm", 1, f32)
        sc2 = tl("sc2", 320)
        p8 = tl("p8", 640)
        f123 = tl("f123", 1440)
        cab = tl("cab", 1280)
        c01 = tl("c01", 960)
        ccd = tl("ccd", 2560)
        c23 = tl("c23", 2560)
        t_e = tl("t_e", 768 * 4)
        t_o = tl("t_o", 768 * 4)
        t0s = tl("t0s", 48)
        t19 = tl("t19", 768)
        r1 = tl("r1", 384)
        r2 = tl("r2", 192)
        r3 = tl("r3", 96)
        s19f = tl("s19f", 64)
        zt = tl("zt", 960, f32)
        out0 = tl("out0", 64, f32)
        outd = tl("outd", 64, f32)
        och = [tl(f"och{i}", ns * 1024, f32) for i, (qb, ns) in enumerate(CHUNKS)]

        def ap(t, off, dims):
            w = t.shape[1]
            return bass.AP(tensor=t.tensor, offset=t.offset + off, ap=[[w, B]] + dims)

        def dram(off, dims):
            return bass.AP(tensor=out_d.tensor, offset=off, ap=[[ROW_F32, B]] + dims)

        # ---- t0 ----------------------------------------------------------
        nc.vector.memset(halfpi[:], float(np.pi / 2))
        nc.scalar.activation(warm[:], halfpi[:], SIN, scale=0.5)
        nc.gpsimd.memset(zt[:], 0.0)
        nc.sync.dma_start(th[:], theta_d)
        nc.sync.dma_start(
            dram(19 * 1024 + 4, [[64, 16], [1, 60]]), ap(zt, 0, [[0, 16], [1, 60]])
        )
        nc.sync.dma_start(dram(64, [[1, 960]]), zt[:])
        nc.vector.tensor_scalar_mul(negh[:], th[:], -0.5)
        nc.vector.scalar_tensor_tensor(
            absv[:], th[:], 0.5, negh[:], MUL, mybir.AluOpType.max
        )
        nc.scalar.activation(ap(sc2, 160, [[1, 160]]), th[:], SIN, scale=0.5)
        nc.scalar.activation(
            ap(sc2, 0, [[1, 160]]), absv[:], SIN, bias=halfpi[:], scale=-1.0
        )

        Z = {"cc": 0, "sc": 80, "cs": 160, "ss": 240, "-sc": 400, "-cs": 480}
        F0B = {("re", 0): (Z["cc"], 160), ("re", 1): (Z["cs"], -160),
               ("im", 0): (Z["-sc"], -160), ("im", 1): (Z["ss"], 160)}

        def emit_p8(q0, w, eng):
            # two ops, one per g0 half (cos-zones cc/sc, sin-zones cs/ss)
            for g0 in (0, 1):
                eng.tensor_tensor(
                    ap(p8, g0 * 160 + q0, [[80, 2], [20, 4], [1, w]]),
                    ap(sc2, g0 * 160 + q0, [[0, 2], [40, 4], [1, w]]),
                    ap(sc2, q0 + 20, [[160, 2], [40, 4], [1, w]]),
                    MUL,
                )
            eng.tensor_scalar_mul(
                ap(p8, 400 + q0, [[80, 2], [20, 4], [1, w]]),
                ap(p8, 80 + q0, [[80, 2], [20, 4], [1, w]]),
                -1.0,
            )

        def emit_tables(q0, w, eng):
            # 8 l-folded copies (one per dest slot) + one nim negation
            for plane, zr0, zr1 in (
                (0, (Z["cc"], Z["-cs"]), (Z["cs"], Z["cc"])),
                (160, (Z["-sc"], Z["ss"]), (Z["ss"], Z["sc"])),
            ):
                for slot, (zsrc, zstp) in (
                    (0, zr0), (120, zr0), (40, zr1), (80, zr1)
                ):
                    eng.tensor_copy(
                        ap(f123, plane + slot + q0, [[480, 3], [20, 2], [1, w]]),
                        ap(p8, zsrc + 20 + q0,
                           [[20, 3], [zstp - zsrc, 2], [1, w]]),
                    )
            eng.tensor_scalar_mul(
                ap(f123, 320 + q0, [[480, 3], [20, 8], [1, w]]),
                ap(f123, 160 + q0, [[480, 3], [20, 8], [1, w]]),
                -1.0,
            )

        def emit_c01(q0, w, eng):
            # 8 mults per (m0, k, t), nesting (m1, a1, a0, q)
            # t=0 (re): k0 = F0re*F1re, k1 = F0im*F1nim
            # t=1 (im): k0 = F0re*F1im, k1 = F0im*F1re
            F1P = {(0, 0): 0, (0, 1): 320, (1, 0): 160, (1, 1): 0}
            for m0 in (0, 1):
                for t in (0, 1):
                    for kpl, fp in ((0, "re"), (1, "im")):
                        b0, s0 = F0B[(fp, m0)]
                        eng.tensor_tensor(
                            ap(cab, t * 640 + kpl * 320 + m0 * 40 + q0,
                               [[20, 2], [80, 4], [1, w]]),
                            ap(p8, b0 + q0, [[0, 4], [s0, 2], [1, w]]),
                            ap(f123, F1P[(t, kpl)] + q0, [[20, 8], [1, w]]),
                            MUL,
                        )
            eng.tensor_add(
                ap(c01, q0, [[320, 2], [20, 16], [1, w]]),
                ap(cab, q0, [[640, 2], [20, 16], [1, w]]),
                ap(cab, 320 + q0, [[640, 2], [20, 16], [1, w]]),
            )
            eng.tensor_add(
                ap(c01, 640 + q0, [[20, 16], [1, w]]),
                ap(c01, q0, [[20, 16], [1, w]]),
                ap(c01, 320 + q0, [[20, 16], [1, w]]),
            )

        def emit_c23(q0, w, eng):
            # 16 mults per (kpair, k, m2, m3), nesting (a3, a2, a1, q)
            F2, F3 = 480, 960
            F3P = ((0, 320), (160, 0))
            for kpair in (0, 1):
                for k in (0, 1):
                    for m2 in (0, 1):
                        for m3 in (0, 1):
                            eng.tensor_tensor(
                                ap(ccd, kpair * 1280 + k * 640 + m2 * 320
                                   + m3 * 160 + q0,
                                   [[20, 8], [1, w]]),
                                ap(f123, F2 + k * 160 + m2 * 80 + q0,
                                   [[0, 2], [20, 4], [1, w]]),
                                ap(f123, F3 + F3P[kpair][k] + m3 * 80 + q0,
                                   [[20, 4], [0, 2], [1, w]]),
                                MUL,
                            )
            # folded add: re @1280, imt @1920
            eng.tensor_add(
                ap(c23, 1280 + q0, [[640, 2], [20, 32], [1, w]]),
                ap(ccd, q0, [[1280, 2], [20, 32], [1, w]]),
                ap(ccd, 640 + q0, [[1280, 2], [20, 32], [1, w]]),
            )
            d32 = [[20, 32], [1, w]]
            eng.tensor_sub(ap(c23, q0, d32), ap(c23, 1280 + q0, d32),
                           ap(c23, 1920 + q0, d32))
            eng.tensor_add(ap(c23, 640 + q0, d32), ap(c23, 1280 + q0, d32),
                           ap(c23, 1920 + q0, d32))

        def emit_chunk(ci, eng_m):
            qb, ns = CHUNKS[ci]
            o = och[ci]
            ts = t_e if ci % 2 == 0 else t_o
            K = 256 * ns
            # 6 mults per (k, a1): nesting (a0, m01, m23, a3, a2, site);
            # T out is contiguous, c01 merges to [[20,8],[0,16],[1,ns]],
            # c23 to [[0,8],[40,16],[1,ns]]
            for k in (0, 1, 2):
                for a1 in (0, 1):
                    eng_m.tensor_tensor(
                        ap(ts, k * K + a1 * 128 * ns,
                           [[64 * ns, 2], [16 * ns, 4], [4 * ns, 4],
                            [2 * ns, 2], [ns, 2], [1, ns]]),
                        ap(c01, k * 320 + a1 * 160 + qb,
                           [[80, 2], [20, 4], [0, 16], [1, ns]]),
                        ap(c23, k * 640 + a1 * 20 + qb,
                           [[0, 8], [160, 4], [80, 2], [40, 2], [1, ns]]),
                        MUL,
                    )
            # 16 subs per (a1, a2, a3, re/im): nesting (site, m01, m23, a0)
            for a1 in (0, 1):
                for a2 in (0, 1):
                    for a3 in (0, 1):
                        eng_s = (nc.vector if SUBENG[ci][a1 * 2 + a3] == "v"
                                 else nc.gpsimd)
                        base = a1 * 128 * ns + a3 * 2 * ns + a2 * ns
                        ob = a1 * 16 + a2 * 8 + a3 * 6
                        sdim = [[1, ns], [16 * ns, 4], [4 * ns, 4], [64 * ns, 2]]
                        odim = [[1024, ns], [256, 4], [64, 4], [32, 2]]
                        eng_s.tensor_sub(
                            ap(o, ob, odim),
                            ap(ts, 2 * K + base, sdim),
                            ap(ts, K + base, sdim),
                        )
                        eng_s.tensor_sub(
                            ap(o, ob + 1, odim),
                            ap(ts, 2 * K + base, sdim),
                            ap(ts, base, sdim),
                        )

        def emit_holes(ci):
            qb, ns = CHUNKS[ci]
            nc.scalar.copy(
                ap(och[ci], 2, [[8, 128 * ns], [1, 4]]),
                ap(zt, 0, [[0, 128 * ns], [1, 4]]),
            )

        # ======== Pool: slice-2 prologues ==================================
        emit_p8(12, 8, nc.gpsimd)
        emit_tables(12, 8, nc.gpsimd)
        emit_p8(8, 4, nc.gpsimd)
        emit_tables(8, 4, nc.gpsimd)

        # ======== DVE: slice 1 (q in [0,8)) ================================
        emit_p8(0, 8, nc.vector)
        emit_tables(0, 8, nc.vector)
        emit_c01(0, 8, nc.vector)
        emit_c23(0, 8, nc.vector)

        # site 0 (DVE): 6 mults per (k, a1), nesting (a0, a2, a3); one sub/add
        for k in (0, 1, 2):
            for a1 in (0, 1):
                nc.vector.tensor_tensor(
                    ap(t0s, k * 16 + a1 * 4, [[8, 2], [2, 2], [1, 2]]),
                    ap(c01, k * 320 + a1 * 160, [[80, 2], [0, 4]]),
                    ap(c23, k * 640 + a1 * 20, [[0, 2], [40, 2], [80, 2]]),
                    MUL,
                )
        nc.vector.tensor_sub(
            ap(out0, 0, [[32, 2], [16, 2], [8, 2], [6, 2], [1, 2]]),
            ap(t0s, 32, [[8, 2], [4, 2], [2, 2], [1, 2], [0, 2]]),
            ap(t0s, 16, [[8, 2], [4, 2], [2, 2], [1, 2], [-16, 2]]),
        )
        nc.scalar.copy(ap(out0, 2, [[8, 8], [1, 4]]), ap(zt, 0, [[0, 8], [1, 4]]))

        emit_holes(0)
        emit_chunk(0, nc.vector)
        nc.sync.dma_start(out_d[:, 1024:3072], och[0][:])
        nc.sync.dma_start(dram(0, [[1, 64]]), out0[:])

        for ci in (1, 2):
            with tc.tile_wait_until(ci):
                emit_holes(ci)
                emit_chunk(ci, nc.vector)
                qb, ns = CHUNKS[ci]
                nc.sync.dma_start(out_d[:, qb * 1024 : (qb + ns) * 1024], och[ci][:])

        # ======== Pool: slice-2 c01/c23 (c23-s2b on DVE below) =============
        emit_c01(8, 4, nc.gpsimd)
        emit_c23(8, 4, nc.gpsimd)
        emit_c01(12, 8, nc.gpsimd)

        # ======== DVE: chunk D, then c23 for q in [12,20) ==================
        with tc.tile_wait_until(3):
            emit_holes(3)
            emit_chunk(3, nc.vector)
        with tc.tile_wait_until(4):
            emit_c23(12, 8, nc.vector)

        tc.tile_set_cur_wait(4)
        # site 19 (DVE): one mult per k (contiguous t19 block), then reduce
        # a0 (stride 64), a1 (stride 128->64), a2 (stride 1), then combine
        for k in (0, 1, 2):
            nc.vector.tensor_tensor(
                ap(t19, k * 256,
                   [[128, 2], [64, 2], [16, 4], [4, 4], [2, 2], [1, 2]]),
                ap(c01, k * 320 + 19, [[160, 2], [80, 2], [20, 4], [0, 16]]),
                ap(c23, k * 640 + 19,
                   [[20, 2], [0, 8], [160, 4], [80, 2], [40, 2]]),
                MUL,
            )
        nc.vector.tensor_add(
            ap(r1, 0, [[128, 3], [64, 2], [1, 64]]),
            ap(t19, 0, [[256, 3], [128, 2], [1, 64]]),
            ap(t19, 64, [[256, 3], [128, 2], [1, 64]]),
        )
        nc.vector.tensor_add(
            ap(r2, 0, [[64, 3], [1, 64]]),
            ap(r1, 0, [[128, 3], [1, 64]]),
            ap(r1, 64, [[128, 3], [1, 64]]),
        )
        nc.vector.tensor_add(
            ap(r3, 0, [[32, 3], [1, 32]]),
            ap(r2, 0, [[64, 3], [2, 32]]),
            ap(r2, 1, [[64, 3], [2, 32]]),
        )
        nc.vector.tensor_sub(
            ap(s19f, 0, [[4, 16], [2, 2], [1, 2]]),
            ap(r3, 64, [[2, 16], [1, 2], [0, 2]]),
            ap(r3, 32, [[2, 16], [1, 2], [-32, 2]]),
        )
        nc.scalar.copy(outd[:], s19f[:])

        qb, ns = CHUNKS[3]
        nc.sync.dma_start(
            dram(19 * 1024, [[64, 16], [1, 4]]), ap(outd, 0, [[4, 16], [1, 4]])
        )
        nc.sync.dma_start(out_d[:, qb * 1024 : (qb + ns) * 1024], och[3][:])

        for ci in (4, 5):
            with tc.tile_wait_until(ci + 1):
                emit_holes(ci)
                emit_chunk(ci, nc.vector)
                qb, ns = CHUNKS[ci]
                nc.sync.dma_start(out_d[:, qb * 1024 : (qb + ns) * 1024], och[ci][:])

    nc.compile()
    return nc


def _get_nc():
    if "nc" not in _CACHE:
        _CACHE["nc"] = _build_nc()
    return _CACHE["nc"]


def kernel(theta, batch_size):
    from concourse.bass_utils import run_bass_kernel_spmd

    theta = np.ascontiguousarray(np.asarray(theta), dtype=np.float32)
    assert theta.shape == (B_TOTAL, P_COLS)
    nc = _get_nc()
    in_maps = [{"theta": theta[c * B : (c + 1) * B]} for c in range(N_CORES)]
    res = run_bass_kernel_spmd(nc, in_maps, core_ids=list(range(N_CORES)))
    _CACHE["last_res"] = res
    full = np.concatenate([r["out"] for r in res.results], axis=0)
    return full.view(np.complex64).reshape(B_TOTAL, NQ, 16, 16, 2)


# revision 7
# speedup vs baseline: 1.3528x; 1.0832x over previous
"""Trainium2 Bass kernel for the batched MPS quantum-circuit forward pass (v6).

Math identical to v3-v5 (Gauss 3-mult complex products, fp16 intermediates,
q-innermost layouts). v6 restructures every op so each OPERAND has at most
3 free dims after AP optimization (hardware TENSOR3D codegen limit), using
index orders chosen so contiguity merges collapse the emitted dims:

  sc2:  cos@0, sin@160; col = l*40 + g*20 + q
  p8:   zone*80 + l*20 + q; zones cc sc cs ss / -sc@400 -cs@480
  f123: l'*480 + plane*160 + m*80 + a*40 + c*20 + q (planes re, im, nim)
  cab:  t*640 + k*320 + idx01*20 + q
  c01:  plane*320 + idx01*20 + q, idx01 = a1*8 + a0*4 + m0*2 + m1
        (planes re, im, S = re+im)
  ccd:  kpair*1280 + k*640 + idx23*20 + q
  c23:  plane*640 + idx23*20 + q, idx23 = m2*16 + m3*8 + a3*4 + a2*2 + a1
        (planes P2' = re-im, P3 = re+im, re, imt)
  T:    k*256ns + a1*128ns + a0*64ns + m01*16ns + m23*4ns + a3*2ns + a2*ns + s
  t0s:  k*16 + a0*8 + a1*4 + a2*2 + a3
  t19:  k*256 + a1*128 + a0*64 + m01*16 + m23*4 + a3*2 + a2
"""

import sys

sys.path.insert(0, "/opt/trn_rl_repo")

import numpy as np

B_TOTAL = 1024
N_CORES = 8
B = B_TOTAL // N_CORES
NQ = 20
P_COLS = 160
ROW_F32 = NQ * 16 * 16 * 2 * 2

_CACHE = {}

CHUNKS = [(1, 2), (3, 2), (5, 3), (8, 3), (11, 4), (15, 4)]
# engine for chunk sub-ops, one char per (a1,a3) quarter: v=DVE g=Pool
SUBENG = ["vvvv", "vvvv", "vvvv", "vvvv", "vvgg", "vvgg"]


def _build_nc():
    import concourse.bass as bass
    import concourse.tile as tile
    from concourse import bacc, mybir

    f32 = mybir.dt.float32
    f16 = mybir.dt.float16
    MUL = mybir.AluOpType.mult
    SIN = mybir.ActivationFunctionType.Sin

    nc = bacc.Bacc("TRN2", target_bir_lowering=False, debug=False)
    theta_d = nc.dram_tensor("theta", [B, P_COLS], f32, kind="ExternalInput").ap()
    out_d = nc.dram_tensor("out", [B, ROW_F32], f32, kind="ExternalOutput").ap()

    from contextlib import ExitStack

    with tile.TileContext(nc) as tc, ExitStack() as ctx:
        pool = ctx.enter_context(tc.tile_pool(name="main", bufs=1))

        def tl(name, w, dt=f16):
            return pool.tile([B, w], dt, name=name)

        th = tl("th", 160, f32)
        absv = tl("absv", 160, f32)
        negh = tl("negh", 160, f32)
        halfpi = tl("halfpi", 1, f32)
        warm = tl("warm", 1, f32)
        sc2 = tl("sc2", 320)
        p8 = tl("p8", 640)
        f123 = tl("f123", 1440)
        cab = tl("cab", 1280)
        c01 = tl("c01", 960)
        ccd = tl("ccd", 2560)
        c23 = tl("c23", 2560)
        t_e = tl("t_e", 768 * 5)
        t_o = tl("t_o", 768 * 5)
        t0s = tl("t0s", 48)
        t19 = tl("t19", 768)
        r1 = tl("r1", 384)
        r2 = tl("r2", 192)
        r3 = tl("r3", 96)
        s19f = tl("s19f", 64)
        zt = tl("zt", 960, f32)
        out0 = tl("out0", 64, f32)
        outd = tl("outd", 64, f32)
        och = [tl(f"och{i}", ns * 1024, f32) for i, (qb, ns) in enumerate(CHUNKS)]

        def ap(t, off, dims):
            w = t.shape[1]
            return bass.AP(tensor=t.tensor, offset=t.offset + off, ap=[[w, B]] + dims)

        def dram(off, dims):
            return bass.AP(tensor=out_d.tensor, offset=off, ap=[[ROW_F32, B]] + dims)

        # ---- t0 ----------------------------------------------------------
        nc.vector.memset(halfpi[:], float(np.pi / 2))
        nc.scalar.activation(warm[:], halfpi[:], SIN, scale=0.5)
        nc.gpsimd.memset(zt[:], 0.0)
        nc.sync.dma_start(th[:], theta_d)
        nc.sync.dma_start(
            dram(19 * 1024 + 4, [[64, 16], [1, 60]]), ap(zt, 0, [[0, 16], [1, 60]])
        )
        nc.sync.dma_start(dram(64, [[1, 960]]), zt[:])
        nc.vector.tensor_scalar_mul(negh[:], th[:], -0.5)
        nc.vector.scalar_tensor_tensor(
            absv[:], th[:], 0.5, negh[:], MUL, mybir.AluOpType.max
        )
        nc.scalar.activation(ap(sc2, 160, [[1, 160]]), th[:], SIN, scale=0.5)
        nc.scalar.activation(
            ap(sc2, 0, [[1, 160]]), absv[:], SIN, bias=halfpi[:], scale=-1.0
        )

        Z = {"cc": 0, "sc": 80, "cs": 160, "ss": 240, "-sc": 400, "-cs": 480}
        F0B = {("re", 0): (Z["cc"], 160), ("re", 1): (Z["cs"], -160),
               ("im", 0): (Z["-sc"], -160), ("im", 1): (Z["ss"], 160)}

        def emit_p8(q0, w, eng):
            # two ops, one per g0 half (cos-zones cc/sc, sin-zones cs/ss)
            for g0 in (0, 1):
                eng.tensor_tensor(
                    ap(p8, g0 * 160 + q0, [[80, 2], [20, 4], [1, w]]),
                    ap(sc2, g0 * 160 + q0, [[0, 2], [40, 4], [1, w]]),
                    ap(sc2, q0 + 20, [[160, 2], [40, 4], [1, w]]),
                    MUL,
                )
            eng.tensor_scalar_mul(
                ap(p8, 400 + q0, [[80, 2], [20, 4], [1, w]]),
                ap(p8, 80 + q0, [[80, 2], [20, 4], [1, w]]),
                -1.0,
            )

        def emit_tables(q0, w, eng):
            # 8 l-folded copies (one per dest slot) + one nim negation
            for plane, zr0, zr1 in (
                (0, (Z["cc"], Z["-cs"]), (Z["cs"], Z["cc"])),
                (160, (Z["-sc"], Z["ss"]), (Z["ss"], Z["sc"])),
            ):
                for slot, (zsrc, zstp) in (
                    (0, zr0), (120, zr0), (40, zr1), (80, zr1)
                ):
                    eng.tensor_copy(
                        ap(f123, plane + slot + q0, [[480, 3], [20, 2], [1, w]]),
                        ap(p8, zsrc + 20 + q0,
                           [[20, 3], [zstp - zsrc, 2], [1, w]]),
                    )
            eng.tensor_scalar_mul(
                ap(f123, 320 + q0, [[480, 3], [20, 8], [1, w]]),
                ap(f123, 160 + q0, [[480, 3], [20, 8], [1, w]]),
                -1.0,
            )

        def emit_c01(q0, w, eng):
            # 8 mults per (m0, k, t), nesting (m1, a1, a0, q)
            # t=0 (re): k0 = F0re*F1re, k1 = F0im*F1nim
            # t=1 (im): k0 = F0re*F1im, k1 = F0im*F1re
            F1P = {(0, 0): 0, (0, 1): 320, (1, 0): 160, (1, 1): 0}
            for m0 in (0, 1):
                for t in (0, 1):
                    for kpl, fp in ((0, "re"), (1, "im")):
                        b0, s0 = F0B[(fp, m0)]
                        eng.tensor_tensor(
                            ap(cab, t * 640 + kpl * 320 + m0 * 40 + q0,
                               [[20, 2], [80, 4], [1, w]]),
                            ap(p8, b0 + q0, [[0, 4], [s0, 2], [1, w]]),
                            ap(f123, F1P[(t, kpl)] + q0, [[20, 8], [1, w]]),
                            MUL,
                        )
            eng.tensor_add(
                ap(c01, q0, [[320, 2], [20, 16], [1, w]]),
                ap(cab, q0, [[640, 2], [20, 16], [1, w]]),
                ap(cab, 320 + q0, [[640, 2], [20, 16], [1, w]]),
            )
            eng.tensor_add(
                ap(c01, 640 + q0, [[20, 16], [1, w]]),
                ap(c01, q0, [[20, 16], [1, w]]),
                ap(c01, 320 + q0, [[20, 16], [1, w]]),
            )

        def emit_c23(q0, w, eng):
            # 8 mults per (kpair, k, m2), nesting (m3, a3, a2, a1, q)
            F2, F3 = 480, 960
            F3P = ((0, 320), (160, 0))
            for kpair in (0, 1):
                for k in (0, 1):
                    for m2 in (0, 1):
                        eng.tensor_tensor(
                            ap(ccd, kpair * 1280 + k * 640 + m2 * 320 + q0,
                               [[20, 16], [1, w]]),
                            ap(f123, F2 + k * 160 + m2 * 80 + q0,
                               [[0, 4], [20, 4], [1, w]]),
                            ap(f123, F3 + F3P[kpair][k] + q0,
                               [[20, 8], [0, 2], [1, w]]),
                            MUL,
                        )
            # folded add: re @1280, imt @1920
            eng.tensor_add(
                ap(c23, 1280 + q0, [[640, 2], [20, 32], [1, w]]),
                ap(ccd, q0, [[1280, 2], [20, 32], [1, w]]),
                ap(ccd, 640 + q0, [[1280, 2], [20, 32], [1, w]]),
            )
            d32 = [[20, 32], [1, w]]
            eng.tensor_sub(ap(c23, q0, d32), ap(c23, 1280 + q0, d32),
                           ap(c23, 1920 + q0, d32))
            eng.tensor_add(ap(c23, 640 + q0, d32), ap(c23, 1280 + q0, d32),
                           ap(c23, 1920 + q0, d32))

        def emit_chunk(ci, eng_m):
            qb, ns = CHUNKS[ci]
            o = och[ci]
            ts = t_e if ci % 2 == 0 else t_o
            K = 256 * ns
            # 6 mults per (k, a1): nesting (a0, m01, m23, a3, a2, site);
            # T out is contiguous, c01 merges to [[20,8],[0,16],[1,ns]],
            # c23 to [[0,8],[40,16],[1,ns]]
            for k in (0, 1, 2):
                for a1 in (0, 1):
                    eng_m.tensor_tensor(
                        ap(ts, k * K + a1 * 128 * ns,
                           [[64 * ns, 2], [16 * ns, 4], [4 * ns, 4],
                            [2 * ns, 2], [ns, 2], [1, ns]]),
                        ap(c01, k * 320 + a1 * 160 + qb,
                           [[80, 2], [20, 4], [0, 16], [1, ns]]),
                        ap(c23, k * 640 + a1 * 20 + qb,
                           [[0, 8], [160, 4], [80, 2], [40, 2], [1, ns]]),
                        MUL,
                    )
            # 8 subs per (a1, a0, re/im): nesting (site, m01, m23, a3, a2);
            # the T side chain-merges to [[1,ns],[ns,64]]
            for a1 in (0, 1):
                for a0 in (0, 1):
                    eng_s = (nc.vector if SUBENG[ci][a1 * 2 + a0] == "v"
                             else nc.gpsimd)
                    base = a1 * 128 * ns + a0 * 64 * ns
                    ob = a1 * 16 + a0 * 32
                    sdim = [[1, ns], [16 * ns, 4], [4 * ns, 4], [2 * ns, 2], [ns, 2]]
                    odim = [[1024, ns], [256, 4], [64, 4], [6, 2], [8, 2]]
                    eng_s.tensor_sub(
                        ap(o, ob, odim),
                        ap(ts, 2 * K + base, sdim),
                        ap(ts, K + base, sdim),
                    )
                    eng_s.tensor_sub(
                        ap(o, ob + 1, odim),
                        ap(ts, 2 * K + base, sdim),
                        ap(ts, base, sdim),
                    )

        def emit_holes(ci):
            qb, ns = CHUNKS[ci]
            nc.scalar.copy(
                ap(och[ci], 2, [[8, 128 * ns], [1, 4]]),
                ap(zt, 0, [[0, 128 * ns], [1, 4]]),
            )

        # ======== Pool: slice-2 prologues ==================================
        emit_p8(12, 8, nc.gpsimd)
        emit_tables(12, 8, nc.gpsimd)
        emit_p8(8, 4, nc.gpsimd)
        emit_tables(8, 4, nc.gpsimd)

        # ======== DVE: slice 1 (q in [0,8)) ================================
        emit_p8(0, 8, nc.vector)
        emit_tables(0, 8, nc.vector)
        emit_c01(0, 8, nc.vector)
        emit_c23(0, 8, nc.vector)

        emit_holes(0)
        emit_chunk(0, nc.vector)
        nc.sync.dma_start(out_d[:, 1024:3072], och[0][:])


        for ci in (1, 2):
            with tc.tile_wait_until(ci):
                emit_holes(ci)
                emit_chunk(ci, nc.vector)
                qb, ns = CHUNKS[ci]
                nc.sync.dma_start(out_d[:, qb * 1024 : (qb + ns) * 1024], och[ci][:])

        # ======== Pool: slice-2 c01/c23 ====================================
        emit_c01(8, 4, nc.gpsimd)
        emit_c23(8, 4, nc.gpsimd)
        emit_c01(12, 8, nc.gpsimd)
        emit_c23(12, 8, nc.gpsimd)

        # ======== DVE: chunk D =============================================
        with tc.tile_wait_until(3):
            emit_holes(3)
            emit_chunk(3, nc.vector)
        tc.tile_set_cur_wait(4)
        # site 19 (DVE): one mult per k (contiguous t19 block), then reduce
        # a0 (stride 64), a1 (stride 128->64), a2 (stride 1), then combine
        for k in (0, 1, 2):
            nc.vector.tensor_tensor(
                ap(t19, k * 256,
                   [[128, 2], [64, 2], [16, 4], [4, 4], [2, 2], [1, 2]]),
                ap(c01, k * 320 + 19, [[160, 2], [80, 2], [20, 4], [0, 16]]),
                ap(c23, k * 640 + 19,
                   [[20, 2], [0, 8], [160, 4], [80, 2], [40, 2]]),
                MUL,
            )
        nc.vector.tensor_add(
            ap(r1, 0, [[128, 3], [1, 128]]),
            ap(t19, 0, [[256, 3], [128, 2], [1, 64]]),
            ap(t19, 64, [[256, 3], [128, 2], [1, 64]]),
        )
        nc.vector.tensor_add(
            ap(r2, 0, [[64, 3], [1, 64]]),
            ap(r1, 0, [[128, 3], [1, 64]]),
            ap(r1, 64, [[128, 3], [1, 64]]),
        )
        nc.vector.tensor_add(
            ap(r3, 0, [[32, 3], [1, 32]]),
            ap(r2, 0, [[64, 3], [2, 32]]),
            ap(r2, 1, [[64, 3], [2, 32]]),
        )
        nc.vector.tensor_sub(
            ap(s19f, 0, [[4, 16], [2, 2], [1, 2]]),
            ap(r3, 64, [[2, 16], [1, 2], [0, 2]]),
            ap(r3, 32, [[2, 16], [1, 2], [-32, 2]]),
        )
        nc.scalar.copy(outd[:], s19f[:])

        for k in (0, 1, 2):
            for a1 in (0, 1):
                nc.vector.tensor_tensor(
                    ap(t0s, k * 16 + a1 * 4, [[8, 2], [2, 2], [1, 2]]),
                    ap(c01, k * 320 + a1 * 160, [[80, 2], [0, 4]]),
                    ap(c23, k * 640 + a1 * 20, [[0, 2], [40, 2], [80, 2]]),
                    MUL,
                )
        nc.vector.tensor_sub(
            ap(out0, 0, [[32, 2], [16, 2], [8, 2], [6, 2], [1, 2]]),
            ap(t0s, 32, [[8, 2], [4, 2], [2, 2], [1, 2], [0, 2]]),
            ap(t0s, 16, [[8, 2], [4, 2], [2, 2], [1, 2], [-16, 2]]),
        )
        nc.scalar.copy(ap(out0, 2, [[8, 8], [1, 4]]), ap(zt, 0, [[0, 8], [1, 4]]))

        qb, ns = CHUNKS[3]
        nc.sync.dma_start(
            dram(19 * 1024, [[64, 16], [1, 4]]), ap(outd, 0, [[4, 16], [1, 4]])
        )
        nc.sync.dma_start(dram(0, [[1, 64]]), out0[:])
        nc.sync.dma_start(out_d[:, qb * 1024 : (qb + ns) * 1024], och[3][:])

        for ci in (4, 5):
            with tc.tile_wait_until(ci + 1):
                emit_holes(ci)
                emit_chunk(ci, nc.vector)
                qb, ns = CHUNKS[ci]
                nc.sync.dma_start(out_d[:, qb * 1024 : (qb + ns) * 1024], och[ci][:])

    nc.compile()
    return nc


def _get_nc():
    if "nc" not in _CACHE:
        _CACHE["nc"] = _build_nc()
    return _CACHE["nc"]


def kernel(theta, batch_size):
    from concourse.bass_utils import run_bass_kernel_spmd

    theta = np.ascontiguousarray(np.asarray(theta), dtype=np.float32)
    assert theta.shape == (B_TOTAL, P_COLS)
    nc = _get_nc()
    in_maps = [{"theta": theta[c * B : (c + 1) * B]} for c in range(N_CORES)]
    res = run_bass_kernel_spmd(nc, in_maps, core_ids=list(range(N_CORES)))
    _CACHE["last_res"] = res
    full = np.concatenate([r["out"] for r in res.results], axis=0)
    return full.view(np.complex64).reshape(B_TOTAL, NQ, 16, 16, 2)


# revision 8
# speedup vs baseline: 1.3660x; 1.0097x over previous
"""Trainium2 Bass kernel for the batched MPS quantum-circuit forward pass (v6).

Math identical to v3-v5 (Gauss 3-mult complex products, fp16 intermediates,
q-innermost layouts). v6 restructures every op so each OPERAND has at most
3 free dims after AP optimization (hardware TENSOR3D codegen limit), using
index orders chosen so contiguity merges collapse the emitted dims:

  sc2:  cos@0, sin@160; col = l*40 + g*20 + q
  p8:   zone*80 + l*20 + q; zones cc sc cs ss / -sc@400 -cs@480
  f123: l'*480 + plane*160 + m*80 + a*40 + c*20 + q (planes re, im, nim)
  cab:  t*640 + k*320 + idx01*20 + q
  c01:  plane*320 + idx01*20 + q, idx01 = a1*8 + a0*4 + m0*2 + m1
        (planes re, im, S = re+im)
  ccd:  kpair*1280 + k*640 + idx23*20 + q
  c23:  plane*640 + idx23*20 + q, idx23 = m2*16 + m3*8 + a3*4 + a2*2 + a1
        (planes P2' = re-im, P3 = re+im, re, imt)
  T:    k*256ns + a1*128ns + a0*64ns + m01*16ns + m23*4ns + a3*2ns + a2*ns + s
  t0s:  k*16 + a0*8 + a1*4 + a2*2 + a3
  t19:  k*256 + a1*128 + a0*64 + m01*16 + m23*4 + a3*2 + a2
"""

import sys

sys.path.insert(0, "/opt/trn_rl_repo")

import numpy as np

B_TOTAL = 1024
N_CORES = 8
B = B_TOTAL // N_CORES
NQ = 20
P_COLS = 160
ROW_F32 = NQ * 16 * 16 * 2 * 2

_CACHE = {}

CHUNKS = [(1, 2), (3, 2), (5, 3), (8, 3), (11, 4), (15, 4)]
# engine for chunk sub-ops, one char per (a1,a3) quarter: v=DVE g=Pool
SUBENG = ["vvvv", "vvvv", "vvvv", "vvgg", "vvgg", "vvgg"]


def _build_nc():
    import concourse.bass as bass
    import concourse.tile as tile
    from concourse import bacc, mybir

    f32 = mybir.dt.float32
    f16 = mybir.dt.float16
    MUL = mybir.AluOpType.mult
    SIN = mybir.ActivationFunctionType.Sin

    nc = bacc.Bacc("TRN2", target_bir_lowering=False, debug=False)
    theta_d = nc.dram_tensor("theta", [B, P_COLS], f32, kind="ExternalInput").ap()
    out_d = nc.dram_tensor("out", [B, ROW_F32], f32, kind="ExternalOutput").ap()

    from contextlib import ExitStack

    with tile.TileContext(nc) as tc, ExitStack() as ctx:
        pool = ctx.enter_context(tc.tile_pool(name="main", bufs=1))

        def tl(name, w, dt=f16):
            return pool.tile([B, w], dt, name=name)

        th = tl("th", 160, f32)
        absv = tl("absv", 160, f32)
        negh = tl("negh", 160, f32)
        halfpi = tl("halfpi", 1, f32)
        warm = tl("warm", 1, f32)
        sc2 = tl("sc2", 320)
        p8 = tl("p8", 640)
        f123 = tl("f123", 1440)
        cab = tl("cab", 1280)
        c01 = tl("c01", 960)
        ccd = tl("ccd", 2560)
        c23 = tl("c23", 2560)
        t_e = tl("t_e", 768 * 5)
        t_o = tl("t_o", 768 * 5)
        t0s = tl("t0s", 48)
        t19 = tl("t19", 768)
        r1 = tl("r1", 384)
        r2 = tl("r2", 192)
        r3 = tl("r3", 96)
        s19f = tl("s19f", 64)
        zt = tl("zt", 960, f32)
        out0 = tl("out0", 64, f32)
        outd = tl("outd", 64, f32)
        och = [tl(f"och{i}", ns * 1024, f32) for i, (qb, ns) in enumerate(CHUNKS)]

        def ap(t, off, dims):
            w = t.shape[1]
            return bass.AP(tensor=t.tensor, offset=t.offset + off, ap=[[w, B]] + dims)

        def dram(off, dims):
            return bass.AP(tensor=out_d.tensor, offset=off, ap=[[ROW_F32, B]] + dims)

        # ---- t0 ----------------------------------------------------------
        nc.vector.memset(halfpi[:], float(np.pi / 2))
        nc.scalar.activation(warm[:], halfpi[:], SIN, scale=0.5)
        nc.gpsimd.memset(zt[:], 0.0)
        nc.sync.dma_start(th[:], theta_d)
        nc.sync.dma_start(
            dram(19 * 1024 + 4, [[64, 16], [1, 60]]), ap(zt, 0, [[0, 16], [1, 60]])
        )
        nc.sync.dma_start(dram(64, [[1, 960]]), zt[:])
        nc.vector.tensor_scalar_mul(negh[:], th[:], -0.5)
        nc.vector.scalar_tensor_tensor(
            absv[:], th[:], 0.5, negh[:], MUL, mybir.AluOpType.max
        )
        nc.scalar.activation(ap(sc2, 160, [[1, 160]]), th[:], SIN, scale=0.5)
        nc.scalar.activation(
            ap(sc2, 0, [[1, 160]]), absv[:], SIN, bias=halfpi[:], scale=-1.0
        )

        Z = {"cc": 0, "sc": 80, "cs": 160, "ss": 240, "-sc": 400, "-cs": 480}
        F0B = {("re", 0): (Z["cc"], 160), ("re", 1): (Z["cs"], -160),
               ("im", 0): (Z["-sc"], -160), ("im", 1): (Z["ss"], 160)}

        def emit_p8(q0, w, eng):
            # two ops, one per g0 half (cos-zones cc/sc, sin-zones cs/ss)
            for g0 in (0, 1):
                eng.tensor_tensor(
                    ap(p8, g0 * 160 + q0, [[80, 2], [20, 4], [1, w]]),
                    ap(sc2, g0 * 160 + q0, [[0, 2], [40, 4], [1, w]]),
                    ap(sc2, q0 + 20, [[160, 2], [40, 4], [1, w]]),
                    MUL,
                )
            eng.tensor_scalar_mul(
                ap(p8, 400 + q0, [[80, 2], [20, 4], [1, w]]),
                ap(p8, 80 + q0, [[80, 2], [20, 4], [1, w]]),
                -1.0,
            )

        def emit_tables(q0, w, eng):
            # 8 l-folded copies (one per dest slot) + one nim negation
            for plane, zr0, zr1 in (
                (0, (Z["cc"], Z["-cs"]), (Z["cs"], Z["cc"])),
                (160, (Z["-sc"], Z["ss"]), (Z["ss"], Z["sc"])),
            ):
                for slot, (zsrc, zstp) in (
                    (0, zr0), (120, zr0), (40, zr1), (80, zr1)
                ):
                    eng.tensor_copy(
                        ap(f123, plane + slot + q0, [[480, 3], [20, 2], [1, w]]),
                        ap(p8, zsrc + 20 + q0,
                           [[20, 3], [zstp - zsrc, 2], [1, w]]),
                    )
            eng.tensor_scalar_mul(
                ap(f123, 320 + q0, [[480, 3], [20, 8], [1, w]]),
                ap(f123, 160 + q0, [[480, 3], [20, 8], [1, w]]),
                -1.0,
            )

        def emit_c01(q0, w, eng):
            # 8 mults per (m0, k, t), nesting (m1, a1, a0, q)
            # t=0 (re): k0 = F0re*F1re, k1 = F0im*F1nim
            # t=1 (im): k0 = F0re*F1im, k1 = F0im*F1re
            F1P = {(0, 0): 0, (0, 1): 320, (1, 0): 160, (1, 1): 0}
            for m0 in (0, 1):
                for t in (0, 1):
                    for kpl, fp in ((0, "re"), (1, "im")):
                        b0, s0 = F0B[(fp, m0)]
                        eng.tensor_tensor(
                            ap(cab, t * 640 + kpl * 320 + m0 * 40 + q0,
                               [[20, 2], [80, 4], [1, w]]),
                            ap(p8, b0 + q0, [[0, 4], [s0, 2], [1, w]]),
                            ap(f123, F1P[(t, kpl)] + q0, [[20, 8], [1, w]]),
                            MUL,
                        )
            eng.tensor_add(
                ap(c01, q0, [[320, 2], [20, 16], [1, w]]),
                ap(cab, q0, [[640, 2], [20, 16], [1, w]]),
                ap(cab, 320 + q0, [[640, 2], [20, 16], [1, w]]),
            )
            eng.tensor_add(
                ap(c01, 640 + q0, [[20, 16], [1, w]]),
                ap(c01, q0, [[20, 16], [1, w]]),
                ap(c01, 320 + q0, [[20, 16], [1, w]]),
            )

        def emit_c23(q0, w, eng):
            # 8 mults per (kpair, k, m2), nesting (m3, a3, a2, a1, q)
            F2, F3 = 480, 960
            F3P = ((0, 320), (160, 0))
            for kpair in (0, 1):
                for k in (0, 1):
                    for m2 in (0, 1):
                        eng.tensor_tensor(
                            ap(ccd, kpair * 1280 + k * 640 + m2 * 320 + q0,
                               [[20, 16], [1, w]]),
                            ap(f123, F2 + k * 160 + m2 * 80 + q0,
                               [[0, 4], [20, 4], [1, w]]),
                            ap(f123, F3 + F3P[kpair][k] + q0,
                               [[20, 8], [0, 2], [1, w]]),
                            MUL,
                        )
            # folded add: re @1280, imt @1920
            eng.tensor_add(
                ap(c23, 1280 + q0, [[640, 2], [20, 32], [1, w]]),
                ap(ccd, q0, [[1280, 2], [20, 32], [1, w]]),
                ap(ccd, 640 + q0, [[1280, 2], [20, 32], [1, w]]),
            )
            d32 = [[20, 32], [1, w]]
            eng.tensor_sub(ap(c23, q0, d32), ap(c23, 1280 + q0, d32),
                           ap(c23, 1920 + q0, d32))
            eng.tensor_add(ap(c23, 640 + q0, d32), ap(c23, 1280 + q0, d32),
                           ap(c23, 1920 + q0, d32))

        def emit_chunk(ci, eng_m):
            qb, ns = CHUNKS[ci]
            o = och[ci]
            ts = t_e if ci % 2 == 0 else t_o
            K = 256 * ns
            # 6 mults per (k, a1): nesting (a0, m01, m23, a3, a2, site);
            # T out is contiguous, c01 merges to [[20,8],[0,16],[1,ns]],
            # c23 to [[0,8],[40,16],[1,ns]]
            for k in (0, 1, 2):
                for a1 in (0, 1):
                    eng_m.tensor_tensor(
                        ap(ts, k * K + a1 * 128 * ns,
                           [[64 * ns, 2], [16 * ns, 4], [4 * ns, 4],
                            [2 * ns, 2], [ns, 2], [1, ns]]),
                        ap(c01, k * 320 + a1 * 160 + qb,
                           [[80, 2], [20, 4], [0, 16], [1, ns]]),
                        ap(c23, k * 640 + a1 * 20 + qb,
                           [[0, 8], [160, 4], [80, 2], [40, 2], [1, ns]]),
                        MUL,
                    )
            # 8 subs per (a1, a0, re/im): nesting (site, m01, m23, a3, a2);
            # the T side chain-merges to [[1,ns],[ns,64]]
            for a1 in (0, 1):
                for a0 in (0, 1):
                    eng_s = (nc.vector if SUBENG[ci][a1 * 2 + a0] == "v"
                             else nc.gpsimd)
                    base = a1 * 128 * ns + a0 * 64 * ns
                    ob = a1 * 16 + a0 * 32
                    sdim = [[1, ns], [16 * ns, 4], [4 * ns, 4], [2 * ns, 2], [ns, 2]]
                    odim = [[1024, ns], [256, 4], [64, 4], [6, 2], [8, 2]]
                    eng_s.tensor_sub(
                        ap(o, ob, odim),
                        ap(ts, 2 * K + base, sdim),
                        ap(ts, K + base, sdim),
                    )
                    eng_s.tensor_sub(
                        ap(o, ob + 1, odim),
                        ap(ts, 2 * K + base, sdim),
                        ap(ts, base, sdim),
                    )

        def emit_holes(ci):
            qb, ns = CHUNKS[ci]
            nc.scalar.copy(
                ap(och[ci], 2, [[8, 128 * ns], [1, 4]]),
                ap(zt, 0, [[0, 128 * ns], [1, 4]]),
            )

        # ======== Pool: slice-2 prologue (single slice q in [8,20)) ========
        emit_p8(8, 12, nc.gpsimd)
        emit_tables(8, 12, nc.gpsimd)

        # ======== DVE: slice 1 (q in [0,8)) ================================
        emit_p8(0, 8, nc.vector)
        emit_tables(0, 8, nc.vector)
        emit_c01(0, 8, nc.vector)
        emit_c23(0, 8, nc.vector)

        emit_holes(0)
        emit_chunk(0, nc.vector)
        nc.sync.dma_start(out_d[:, 1024:3072], och[0][:])


        for ci in (1, 2):
            with tc.tile_wait_until(ci):
                emit_holes(ci)
                emit_chunk(ci, nc.vector)
                qb, ns = CHUNKS[ci]
                nc.sync.dma_start(out_d[:, qb * 1024 : (qb + ns) * 1024], och[ci][:])

        # ======== Pool: slice-2 c01/c23 ====================================
        emit_c01(8, 12, nc.gpsimd)
        emit_c23(8, 12, nc.gpsimd)

        # ======== DVE: chunk D =============================================
        with tc.tile_wait_until(3):
            emit_holes(3)
            emit_chunk(3, nc.vector)
            _qb, _ns = CHUNKS[3]
            nc.sync.dma_start(out_d[:, _qb * 1024 : (_qb + _ns) * 1024], och[3][:])
        with tc.tile_wait_until(4):
            emit_holes(4)
            emit_chunk(4, nc.vector)
            _qb, _ns = CHUNKS[4]
            nc.sync.dma_start(out_d[:, _qb * 1024 : (_qb + _ns) * 1024], och[4][:])
        tc.tile_set_cur_wait(5)
        # site 19 (DVE): one mult per k (contiguous t19 block), then reduce
        # a0 (stride 64), a1 (stride 128->64), a2 (stride 1), then combine
        for k in (0, 1, 2):
            nc.vector.tensor_tensor(
                ap(t19, k * 256,
                   [[128, 2], [64, 2], [16, 4], [4, 4], [2, 2], [1, 2]]),
                ap(c01, k * 320 + 19, [[160, 2], [80, 2], [20, 4], [0, 16]]),
                ap(c23, k * 640 + 19,
                   [[20, 2], [0, 8], [160, 4], [80, 2], [40, 2]]),
                MUL,
            )
        nc.vector.tensor_add(
            ap(r1, 0, [[128, 3], [1, 128]]),
            ap(t19, 0, [[256, 3], [128, 2], [1, 64]]),
            ap(t19, 64, [[256, 3], [128, 2], [1, 64]]),
        )
        nc.vector.tensor_add(
            ap(r2, 0, [[64, 3], [1, 64]]),
            ap(r1, 0, [[128, 3], [1, 64]]),
            ap(r1, 64, [[128, 3], [1, 64]]),
        )
        nc.vector.tensor_add(
            ap(r3, 0, [[32, 3], [1, 32]]),
            ap(r2, 0, [[64, 3], [2, 32]]),
            ap(r2, 1, [[64, 3], [2, 32]]),
        )
        nc.vector.tensor_sub(
            ap(s19f, 0, [[4, 16], [2, 2], [1, 2]]),
            ap(r3, 64, [[2, 16], [1, 2], [0, 2]]),
            ap(r3, 32, [[2, 16], [1, 2], [-32, 2]]),
        )
        nc.scalar.copy(outd[:], s19f[:])

        for k in (0, 1, 2):
            for a1 in (0, 1):
                nc.vector.tensor_tensor(
                    ap(t0s, k * 16 + a1 * 4, [[8, 2], [2, 2], [1, 2]]),
                    ap(c01, k * 320 + a1 * 160, [[80, 2], [0, 4]]),
                    ap(c23, k * 640 + a1 * 20, [[0, 2], [40, 2], [80, 2]]),
                    MUL,
                )
        nc.vector.tensor_sub(
            ap(out0, 0, [[32, 2], [16, 2], [8, 2], [6, 2], [1, 2]]),
            ap(t0s, 32, [[8, 2], [4, 2], [2, 2], [1, 2], [0, 2]]),
            ap(t0s, 16, [[8, 2], [4, 2], [2, 2], [1, 2], [-16, 2]]),
        )
        nc.scalar.copy(ap(out0, 2, [[8, 8], [1, 4]]), ap(zt, 0, [[0, 8], [1, 4]]))

        nc.sync.dma_start(
            dram(19 * 1024, [[64, 16], [1, 4]]), ap(outd, 0, [[4, 16], [1, 4]])
        )
        nc.sync.dma_start(dram(0, [[1, 64]]), out0[:])

        with tc.tile_wait_until(6):
            emit_holes(5)
            emit_chunk(5, nc.vector)
            qb, ns = CHUNKS[5]
            nc.sync.dma_start(out_d[:, qb * 1024 : (qb + ns) * 1024], och[5][:])

    nc.compile()
    return nc


def _get_nc():
    if "nc" not in _CACHE:
        _CACHE["nc"] = _build_nc()
    return _CACHE["nc"]


def kernel(theta, batch_size):
    from concourse.bass_utils import run_bass_kernel_spmd

    theta = np.ascontiguousarray(np.asarray(theta), dtype=np.float32)
    assert theta.shape == (B_TOTAL, P_COLS)
    nc = _get_nc()
    in_maps = [{"theta": theta[c * B : (c + 1) * B]} for c in range(N_CORES)]
    res = run_bass_kernel_spmd(nc, in_maps, core_ids=list(range(N_CORES)))
    _CACHE["last_res"] = res
    full = np.concatenate([r["out"] for r in res.results], axis=0)
    return full.view(np.complex64).reshape(B_TOTAL, NQ, 16, 16, 2)


# revision 9
# speedup vs baseline: 1.3686x; 1.0019x over previous
"""Trainium2 Bass kernel for the batched MPS quantum-circuit forward pass (v6).

Math identical to v3-v5 (Gauss 3-mult complex products, fp16 intermediates,
q-innermost layouts). v6 restructures every op so each OPERAND has at most
3 free dims after AP optimization (hardware TENSOR3D codegen limit), using
index orders chosen so contiguity merges collapse the emitted dims:

  sc2:  cos@0, sin@160; col = l*40 + g*20 + q
  p8:   zone*80 + l*20 + q; zones cc sc cs ss / -sc@400 -cs@480
  f123: l'*480 + plane*160 + m*80 + a*40 + c*20 + q (planes re, im, nim)
  cab:  t*640 + k*320 + idx01*20 + q
  c01:  plane*320 + idx01*20 + q, idx01 = a1*8 + a0*4 + m0*2 + m1
        (planes re, im, S = re+im)
  ccd:  kpair*1280 + k*640 + idx23*20 + q
  c23:  plane*640 + idx23*20 + q, idx23 = m2*16 + m3*8 + a3*4 + a2*2 + a1
        (planes P2' = re-im, P3 = re+im, re, imt)
  T:    k*256ns + a1*128ns + a0*64ns + m01*16ns + m23*4ns + a3*2ns + a2*ns + s
  t0s:  k*16 + a0*8 + a1*4 + a2*2 + a3
  t19:  k*256 + a1*128 + a0*64 + m01*16 + m23*4 + a3*2 + a2
"""

import sys

sys.path.insert(0, "/opt/trn_rl_repo")

import numpy as np

B_TOTAL = 1024
N_CORES = 8
B = B_TOTAL // N_CORES
NQ = 20
P_COLS = 160
ROW_F32 = NQ * 16 * 16 * 2 * 2

_CACHE = {}

CHUNKS = [(1, 2), (3, 2), (5, 3), (8, 3), (11, 4), (15, 4)]
# engine for chunk sub-ops, one char per (a1,a3) quarter: v=DVE g=Pool
SUBENG = ["vvvv", "vvvv", "vvvv", "vvgg", "vvgg", "vvgg"]


def _build_nc():
    import concourse.bass as bass
    import concourse.tile as tile
    from concourse import bacc, mybir

    f32 = mybir.dt.float32
    f16 = mybir.dt.float16
    MUL = mybir.AluOpType.mult
    SIN = mybir.ActivationFunctionType.Sin

    nc = bacc.Bacc("TRN2", target_bir_lowering=False, debug=False)
    theta_d = nc.dram_tensor("theta", [B, P_COLS], f32, kind="ExternalInput").ap()
    out_d = nc.dram_tensor("out", [B, ROW_F32], f32, kind="ExternalOutput").ap()

    from contextlib import ExitStack

    with tile.TileContext(nc) as tc, ExitStack() as ctx:
        pool = ctx.enter_context(tc.tile_pool(name="main", bufs=1))

        def tl(name, w, dt=f16):
            return pool.tile([B, w], dt, name=name)

        th = tl("th", 160, f32)
        absv = tl("absv", 160, f32)
        negh = tl("negh", 160, f32)
        halfpi = tl("halfpi", 1, f32)
        warm = tl("warm", 1, f32)
        sc2 = tl("sc2", 320)
        p8 = tl("p8", 640)
        f123 = tl("f123", 1440)
        cab = tl("cab", 1280)
        c01 = tl("c01", 960)
        ccd = tl("ccd", 2560)
        c23 = tl("c23", 2560)
        t_e = tl("t_e", 768 * 5)
        t_o = tl("t_o", 768 * 5)
        t0s = tl("t0s", 48)
        t19 = tl("t19", 768)
        r1 = tl("r1", 384)
        r2 = tl("r2", 192)
        r3 = tl("r3", 96)
        s19f = tl("s19f", 64)
        zt = tl("zt", 960, f32)
        out0 = tl("out0", 64, f32)
        outd = tl("outd", 64, f32)
        och = [tl(f"och{i}", ns * 1024, f32) for i, (qb, ns) in enumerate(CHUNKS)]

        def ap(t, off, dims):
            w = t.shape[1]
            return bass.AP(tensor=t.tensor, offset=t.offset + off, ap=[[w, B]] + dims)

        def dram(off, dims):
            return bass.AP(tensor=out_d.tensor, offset=off, ap=[[ROW_F32, B]] + dims)

        # ---- t0 ----------------------------------------------------------
        nc.vector.memset(halfpi[:], float(np.pi / 2))
        nc.scalar.activation(warm[:], halfpi[:], SIN, scale=0.5)
        nc.gpsimd.memset(zt[:], 0.0)
        nc.sync.dma_start(th[:], theta_d)
        nc.sync.dma_start(
            dram(19 * 1024 + 4, [[64, 16], [1, 60]]), ap(zt, 0, [[0, 16], [1, 60]])
        )
        nc.sync.dma_start(dram(64, [[1, 960]]), zt[:])
        nc.vector.tensor_scalar_mul(negh[:], th[:], -0.5)
        nc.vector.scalar_tensor_tensor(
            absv[:], th[:], 0.5, negh[:], MUL, mybir.AluOpType.max
        )
        # sliced sin/cos: q in [0,8) first (slice-1 consumers), then the rest
        nc.scalar.activation(
            ap(sc2, 160, [[20, 8], [1, 8]]), ap(th, 0, [[20, 8], [1, 8]]),
            SIN, scale=0.5,
        )
        nc.scalar.activation(
            ap(sc2, 0, [[20, 8], [1, 8]]), ap(absv, 0, [[20, 8], [1, 8]]),
            SIN, bias=halfpi[:], scale=-1.0,
        )
        nc.scalar.activation(
            ap(sc2, 168, [[20, 8], [1, 12]]), ap(th, 8, [[20, 8], [1, 12]]),
            SIN, scale=0.5,
        )
        nc.scalar.activation(
            ap(sc2, 8, [[20, 8], [1, 12]]), ap(absv, 8, [[20, 8], [1, 12]]),
            SIN, bias=halfpi[:], scale=-1.0,
        )

        Z = {"cc": 0, "sc": 80, "cs": 160, "ss": 240, "-sc": 400, "-cs": 480}
        F0B = {("re", 0): (Z["cc"], 160), ("re", 1): (Z["cs"], -160),
               ("im", 0): (Z["-sc"], -160), ("im", 1): (Z["ss"], 160)}

        def emit_p8(q0, w, eng):
            # two ops, one per g0 half (cos-zones cc/sc, sin-zones cs/ss)
            for g0 in (0, 1):
                eng.tensor_tensor(
                    ap(p8, g0 * 160 + q0, [[80, 2], [20, 4], [1, w]]),
                    ap(sc2, g0 * 160 + q0, [[0, 2], [40, 4], [1, w]]),
                    ap(sc2, q0 + 20, [[160, 2], [40, 4], [1, w]]),
                    MUL,
                )
            eng.tensor_scalar_mul(
                ap(p8, 400 + q0, [[80, 2], [20, 4], [1, w]]),
                ap(p8, 80 + q0, [[80, 2], [20, 4], [1, w]]),
                -1.0,
            )

        def emit_tables(q0, w, eng):
            # 8 l-folded copies (one per dest slot) + one nim negation
            for plane, zr0, zr1 in (
                (0, (Z["cc"], Z["-cs"]), (Z["cs"], Z["cc"])),
                (160, (Z["-sc"], Z["ss"]), (Z["ss"], Z["sc"])),
            ):
                for slot, (zsrc, zstp) in (
                    (0, zr0), (120, zr0), (40, zr1), (80, zr1)
                ):
                    eng.tensor_copy(
                        ap(f123, plane + slot + q0, [[480, 3], [20, 2], [1, w]]),
                        ap(p8, zsrc + 20 + q0,
                           [[20, 3], [zstp - zsrc, 2], [1, w]]),
                    )
            eng.tensor_scalar_mul(
                ap(f123, 320 + q0, [[480, 3], [20, 8], [1, w]]),
                ap(f123, 160 + q0, [[480, 3], [20, 8], [1, w]]),
                -1.0,
            )

        def emit_c01(q0, w, eng):
            # 8 mults per (m0, k, t), nesting (m1, a1, a0, q)
            # t=0 (re): k0 = F0re*F1re, k1 = F0im*F1nim
            # t=1 (im): k0 = F0re*F1im, k1 = F0im*F1re
            F1P = {(0, 0): 0, (0, 1): 320, (1, 0): 160, (1, 1): 0}
            for m0 in (0, 1):
                for t in (0, 1):
                    for kpl, fp in ((0, "re"), (1, "im")):
                        b0, s0 = F0B[(fp, m0)]
                        eng.tensor_tensor(
                            ap(cab, t * 640 + kpl * 320 + m0 * 40 + q0,
                               [[20, 2], [80, 4], [1, w]]),
                            ap(p8, b0 + q0, [[0, 4], [s0, 2], [1, w]]),
                            ap(f123, F1P[(t, kpl)] + q0, [[20, 8], [1, w]]),
                            MUL,
                        )
            eng.tensor_add(
                ap(c01, q0, [[320, 2], [20, 16], [1, w]]),
                ap(cab, q0, [[640, 2], [20, 16], [1, w]]),
                ap(cab, 320 + q0, [[640, 2], [20, 16], [1, w]]),
            )
            eng.tensor_add(
                ap(c01, 640 + q0, [[20, 16], [1, w]]),
                ap(c01, q0, [[20, 16], [1, w]]),
                ap(c01, 320 + q0, [[20, 16], [1, w]]),
            )

        def emit_c23(q0, w, eng):
            # 8 mults per (kpair, k, m2), nesting (m3, a3, a2, a1, q)
            F2, F3 = 480, 960
            F3P = ((0, 320), (160, 0))
            for kpair in (0, 1):
                for k in (0, 1):
                    for m2 in (0, 1):
                        eng.tensor_tensor(
                            ap(ccd, kpair * 1280 + k * 640 + m2 * 320 + q0,
                               [[20, 16], [1, w]]),
                            ap(f123, F2 + k * 160 + m2 * 80 + q0,
                               [[0, 4], [20, 4], [1, w]]),
                            ap(f123, F3 + F3P[kpair][k] + q0,
                               [[20, 8], [0, 2], [1, w]]),
                            MUL,
                        )
            # folded add: re @1280, imt @1920
            eng.tensor_add(
                ap(c23, 1280 + q0, [[640, 2], [20, 32], [1, w]]),
                ap(ccd, q0, [[1280, 2], [20, 32], [1, w]]),
                ap(ccd, 640 + q0, [[1280, 2], [20, 32], [1, w]]),
            )
            d32 = [[20, 32], [1, w]]
            eng.tensor_sub(ap(c23, q0, d32), ap(c23, 1280 + q0, d32),
                           ap(c23, 1920 + q0, d32))
            eng.tensor_add(ap(c23, 640 + q0, d32), ap(c23, 1280 + q0, d32),
                           ap(c23, 1920 + q0, d32))

        def emit_chunk(ci, eng_m):
            qb, ns = CHUNKS[ci]
            o = och[ci]
            ts = t_e if ci % 2 == 0 else t_o
            K = 256 * ns
            # 6 mults per (k, a1): nesting (a0, m01, m23, a3, a2, site);
            # T out is contiguous, c01 merges to [[20,8],[0,16],[1,ns]],
            # c23 to [[0,8],[40,16],[1,ns]]
            for k in (0, 1, 2):
                for a1 in (0, 1):
                    eng_m.tensor_tensor(
                        ap(ts, k * K + a1 * 128 * ns,
                           [[64 * ns, 2], [16 * ns, 4], [4 * ns, 4],
                            [2 * ns, 2], [ns, 2], [1, ns]]),
                        ap(c01, k * 320 + a1 * 160 + qb,
                           [[80, 2], [20, 4], [0, 16], [1, ns]]),
                        ap(c23, k * 640 + a1 * 20 + qb,
                           [[0, 8], [160, 4], [80, 2], [40, 2], [1, ns]]),
                        MUL,
                    )
            # 8 subs per (a1, a0, re/im): nesting (site, m01, m23, a3, a2);
            # the T side chain-merges to [[1,ns],[ns,64]]
            for a1 in (0, 1):
                for a0 in (0, 1):
                    eng_s = (nc.vector if SUBENG[ci][a1 * 2 + a0] == "v"
                             else nc.gpsimd)
                    base = a1 * 128 * ns + a0 * 64 * ns
                    ob = a1 * 16 + a0 * 32
                    sdim = [[1, ns], [16 * ns, 4], [4 * ns, 4], [2 * ns, 2], [ns, 2]]
                    odim = [[1024, ns], [256, 4], [64, 4], [6, 2], [8, 2]]
                    eng_s.tensor_sub(
                        ap(o, ob, odim),
                        ap(ts, 2 * K + base, sdim),
                        ap(ts, K + base, sdim),
                    )
                    eng_s.tensor_sub(
                        ap(o, ob + 1, odim),
                        ap(ts, 2 * K + base, sdim),
                        ap(ts, base, sdim),
                    )

        def emit_holes(ci):
            qb, ns = CHUNKS[ci]
            nc.scalar.copy(
                ap(och[ci], 2, [[8, 128 * ns], [1, 4]]),
                ap(zt, 0, [[0, 128 * ns], [1, 4]]),
            )

        # ======== Pool: slice-2 prologue (single slice q in [8,20)) ========
        emit_p8(8, 12, nc.gpsimd)
        emit_tables(8, 12, nc.gpsimd)

        # ======== DVE: slice 1 (q in [0,8)) ================================
        emit_p8(0, 8, nc.vector)
        emit_tables(0, 8, nc.vector)
        emit_c01(0, 8, nc.vector)
        emit_c23(0, 8, nc.vector)

        emit_holes(0)
        emit_chunk(0, nc.vector)
        nc.sync.dma_start(out_d[:, 1024:3072], och[0][:])


        for ci in (1, 2):
            with tc.tile_wait_until(ci):
                emit_holes(ci)
                emit_chunk(ci, nc.vector)
                qb, ns = CHUNKS[ci]
                nc.sync.dma_start(out_d[:, qb * 1024 : (qb + ns) * 1024], och[ci][:])

        # ======== Pool: slice-2 c01/c23 ====================================
        emit_c01(8, 12, nc.gpsimd)
        emit_c23(8, 12, nc.gpsimd)

        # ======== DVE: chunk D =============================================
        with tc.tile_wait_until(3):
            emit_holes(3)
            emit_chunk(3, nc.vector)
            _qb, _ns = CHUNKS[3]
            nc.sync.dma_start(out_d[:, _qb * 1024 : (_qb + _ns) * 1024], och[3][:])
        with tc.tile_wait_until(4):
            emit_holes(4)
            emit_chunk(4, nc.vector)
            _qb, _ns = CHUNKS[4]
            nc.sync.dma_start(out_d[:, _qb * 1024 : (_qb + _ns) * 1024], och[4][:])
        tc.tile_set_cur_wait(5)
        # site 19 (DVE): one mult per k (contiguous t19 block), then reduce
        # a0 (stride 64), a1 (stride 128->64), a2 (stride 1), then combine
        for k in (0, 1, 2):
            nc.vector.tensor_tensor(
                ap(t19, k * 256,
                   [[128, 2], [64, 2], [16, 4], [4, 4], [2, 2], [1, 2]]),
                ap(c01, k * 320 + 19, [[160, 2], [80, 2], [20, 4], [0, 16]]),
                ap(c23, k * 640 + 19,
                   [[20, 2], [0, 8], [160, 4], [80, 2], [40, 2]]),
                MUL,
            )
        nc.vector.tensor_add(
            ap(r1, 0, [[128, 3], [1, 128]]),
            ap(t19, 0, [[256, 3], [128, 2], [1, 64]]),
            ap(t19, 64, [[256, 3], [128, 2], [1, 64]]),
        )
        nc.vector.tensor_add(
            ap(r2, 0, [[64, 3], [1, 64]]),
            ap(r1, 0, [[128, 3], [1, 64]]),
            ap(r1, 64, [[128, 3], [1, 64]]),
        )
        nc.vector.tensor_add(
            ap(r3, 0, [[32, 3], [1, 32]]),
            ap(r2, 0, [[64, 3], [2, 32]]),
            ap(r2, 1, [[64, 3], [2, 32]]),
        )
        nc.vector.tensor_sub(
            ap(s19f, 0, [[4, 16], [2, 2], [1, 2]]),
            ap(r3, 64, [[2, 16], [1, 2], [0, 2]]),
            ap(r3, 32, [[2, 16], [1, 2], [-32, 2]]),
        )
        nc.scalar.copy(outd[:], s19f[:])

        for k in (0, 1, 2):
            for a1 in (0, 1):
                nc.vector.tensor_tensor(
                    ap(t0s, k * 16 + a1 * 4, [[8, 2], [2, 2], [1, 2]]),
                    ap(c01, k * 320 + a1 * 160, [[80, 2], [0, 4]]),
                    ap(c23, k * 640 + a1 * 20, [[0, 2], [40, 2], [80, 2]]),
                    MUL,
                )
        nc.vector.tensor_sub(
            ap(out0, 0, [[32, 2], [16, 2], [8, 2], [6, 2], [1, 2]]),
            ap(t0s, 32, [[8, 2], [4, 2], [2, 2], [1, 2], [0, 2]]),
            ap(t0s, 16, [[8, 2], [4, 2], [2, 2], [1, 2], [-16, 2]]),
        )
        nc.scalar.copy(ap(out0, 2, [[8, 8], [1, 4]]), ap(zt, 0, [[0, 8], [1, 4]]))

        nc.sync.dma_start(
            dram(19 * 1024, [[64, 16], [1, 4]]), ap(outd, 0, [[4, 16], [1, 4]])
        )
        nc.sync.dma_start(dram(0, [[1, 64]]), out0[:])

        with tc.tile_wait_until(6):
            emit_holes(5)
            emit_chunk(5, nc.vector)
            qb, ns = CHUNKS[5]
            nc.sync.dma_start(out_d[:, qb * 1024 : (qb + ns) * 1024], och[5][:])

    nc.compile()
    return nc


def _get_nc():
    if "nc" not in _CACHE:
        _CACHE["nc"] = _build_nc()
    return _CACHE["nc"]


def kernel(theta, batch_size):
    from concourse.bass_utils import run_bass_kernel_spmd

    theta = np.ascontiguousarray(np.asarray(theta), dtype=np.float32)
    assert theta.shape == (B_TOTAL, P_COLS)
    nc = _get_nc()
    in_maps = [{"theta": theta[c * B : (c + 1) * B]} for c in range(N_CORES)]
    res = run_bass_kernel_spmd(nc, in_maps, core_ids=list(range(N_CORES)))
    _CACHE["last_res"] = res
    full = np.concatenate([r["out"] for r in res.results], axis=0)
    return full.view(np.complex64).reshape(B_TOTAL, NQ, 16, 16, 2)


# revision 10
# speedup vs baseline: 1.3719x; 1.0024x over previous
"""Trainium2 Bass kernel for the batched MPS quantum-circuit forward pass (v6).

Math identical to v3-v5 (Gauss 3-mult complex products, fp16 intermediates,
q-innermost layouts). v6 restructures every op so each OPERAND has at most
3 free dims after AP optimization (hardware TENSOR3D codegen limit), using
index orders chosen so contiguity merges collapse the emitted dims:

  sc2:  cos@0, sin@160; col = l*40 + g*20 + q
  p8:   zone*80 + l*20 + q; zones cc sc cs ss / -sc@400 -cs@480
  f123: l'*480 + plane*160 + m*80 + a*40 + c*20 + q (planes re, im, nim)
  cab:  t*640 + k*320 + idx01*20 + q
  c01:  plane*320 + idx01*20 + q, idx01 = a1*8 + a0*4 + m0*2 + m1
        (planes re, im, S = re+im)
  ccd:  kpair*1280 + k*640 + idx23*20 + q
  c23:  plane*640 + idx23*20 + q, idx23 = m2*16 + m3*8 + a3*4 + a2*2 + a1
        (planes P2' = re-im, P3 = re+im, re, imt)
  T:    k*256ns + a1*128ns + a0*64ns + m01*16ns + m23*4ns + a3*2ns + a2*ns + s
  t0s:  k*16 + a0*8 + a1*4 + a2*2 + a3
  t19:  k*256 + a1*128 + a0*64 + m01*16 + m23*4 + a3*2 + a2
"""

import sys

sys.path.insert(0, "/opt/trn_rl_repo")

import numpy as np

B_TOTAL = 1024
N_CORES = 8
B = B_TOTAL // N_CORES
NQ = 20
P_COLS = 160
ROW_F32 = NQ * 16 * 16 * 2 * 2

_CACHE = {}

CHUNKS = [(1, 2), (3, 2), (5, 3), (8, 3), (11, 4), (15, 4)]
# engine for chunk sub-ops, one char per (a1,a3) quarter: v=DVE g=Pool
SUBENG = ["vvvv", "vvvv", "vvvv", "vvgg", "vvgg", "vvgg"]


def _build_nc():
    import concourse.bass as bass
    import concourse.tile as tile
    from concourse import bacc, mybir

    f32 = mybir.dt.float32
    f16 = mybir.dt.float16
    MUL = mybir.AluOpType.mult
    SIN = mybir.ActivationFunctionType.Sin

    nc = bacc.Bacc("TRN2", target_bir_lowering=False, debug=False)
    theta_d = nc.dram_tensor("theta", [B, P_COLS], f32, kind="ExternalInput").ap()
    out_d = nc.dram_tensor("out", [B, ROW_F32], f32, kind="ExternalOutput").ap()

    from contextlib import ExitStack

    with tile.TileContext(nc) as tc, ExitStack() as ctx:
        pool = ctx.enter_context(tc.tile_pool(name="main", bufs=1))

        def tl(name, w, dt=f16):
            return pool.tile([B, w], dt, name=name)

        th = tl("th", 160, f32)
        absv = tl("absv", 160, f32)
        negh = tl("negh", 160, f32)
        halfpi = tl("halfpi", 1, f32)
        warm = tl("warm", 1, f32)
        sc2 = tl("sc2", 320)
        p8 = tl("p8", 640)
        f123 = tl("f123", 1440)
        cab = tl("cab", 1280)
        c01 = tl("c01", 960)
        ccd = tl("ccd", 2560)
        c23 = tl("c23", 2560)
        t_e = tl("t_e", 768 * 5)
        t_o = tl("t_o", 768 * 5)
        t0s = tl("t0s", 48)
        t19 = tl("t19", 768)
        r1 = tl("r1", 384)
        r2 = tl("r2", 192)
        r3 = tl("r3", 96)
        s19f = tl("s19f", 64)
        zt = tl("zt", 960, f32)
        out0 = tl("out0", 64, f32)
        outd = tl("outd", 64, f32)
        och = [tl(f"och{i}", ns * 1024, f32) for i, (qb, ns) in enumerate(CHUNKS)]

        def ap(t, off, dims):
            w = t.shape[1]
            return bass.AP(tensor=t.tensor, offset=t.offset + off, ap=[[w, B]] + dims)

        def dram(off, dims):
            return bass.AP(tensor=out_d.tensor, offset=off, ap=[[ROW_F32, B]] + dims)

        # ---- t0 ----------------------------------------------------------
        nc.vector.memset(halfpi[:], float(np.pi / 2))
        nc.scalar.activation(warm[:], halfpi[:], SIN, scale=0.5)
        nc.gpsimd.memset(zt[:], 0.0)
        nc.sync.dma_start(th[:], theta_d)
        nc.sync.dma_start(
            dram(19 * 1024 + 4, [[64, 16], [1, 60]]), ap(zt, 0, [[0, 16], [1, 60]])
        )
        nc.sync.dma_start(dram(64, [[1, 960]]), zt[:])
        nc.vector.tensor_scalar_mul(negh[:], th[:], -0.5)
        nc.vector.scalar_tensor_tensor(
            ap(absv, 0, [[20, 8], [1, 8]]),
            ap(th, 0, [[20, 8], [1, 8]]), 0.5,
            ap(negh, 0, [[20, 8], [1, 8]]), MUL, mybir.AluOpType.max,
        )
        nc.vector.scalar_tensor_tensor(
            ap(absv, 8, [[20, 8], [1, 12]]),
            ap(th, 8, [[20, 8], [1, 12]]), 0.5,
            ap(negh, 8, [[20, 8], [1, 12]]), MUL, mybir.AluOpType.max,
        )
        # sliced sin/cos: q in [0,8) first (slice-1 consumers), then the rest
        nc.scalar.activation(
            ap(sc2, 160, [[20, 8], [1, 8]]), ap(th, 0, [[20, 8], [1, 8]]),
            SIN, scale=0.5,
        )
        nc.scalar.activation(
            ap(sc2, 0, [[20, 8], [1, 8]]), ap(absv, 0, [[20, 8], [1, 8]]),
            SIN, bias=halfpi[:], scale=-1.0,
        )
        nc.scalar.activation(
            ap(sc2, 168, [[20, 8], [1, 12]]), ap(th, 8, [[20, 8], [1, 12]]),
            SIN, scale=0.5,
        )
        nc.scalar.activation(
            ap(sc2, 8, [[20, 8], [1, 12]]), ap(absv, 8, [[20, 8], [1, 12]]),
            SIN, bias=halfpi[:], scale=-1.0,
        )

        Z = {"cc": 0, "sc": 80, "cs": 160, "ss": 240, "-sc": 400, "-cs": 480}
        F0B = {("re", 0): (Z["cc"], 160), ("re", 1): (Z["cs"], -160),
               ("im", 0): (Z["-sc"], -160), ("im", 1): (Z["ss"], 160)}

        def emit_p8(q0, w, eng):
            # two ops, one per g0 half (cos-zones cc/sc, sin-zones cs/ss)
            for g0 in (0, 1):
                eng.tensor_tensor(
                    ap(p8, g0 * 160 + q0, [[80, 2], [20, 4], [1, w]]),
                    ap(sc2, g0 * 160 + q0, [[0, 2], [40, 4], [1, w]]),
                    ap(sc2, q0 + 20, [[160, 2], [40, 4], [1, w]]),
                    MUL,
                )
            eng.tensor_scalar_mul(
                ap(p8, 400 + q0, [[80, 2], [20, 4], [1, w]]),
                ap(p8, 80 + q0, [[80, 2], [20, 4], [1, w]]),
                -1.0,
            )

        def emit_tables(q0, w, eng):
            # 8 l-folded copies (one per dest slot) + one nim negation
            for plane, zr0, zr1 in (
                (0, (Z["cc"], Z["-cs"]), (Z["cs"], Z["cc"])),
                (160, (Z["-sc"], Z["ss"]), (Z["ss"], Z["sc"])),
            ):
                for slot, (zsrc, zstp) in (
                    (0, zr0), (120, zr0), (40, zr1), (80, zr1)
                ):
                    eng.tensor_copy(
                        ap(f123, plane + slot + q0, [[480, 3], [20, 2], [1, w]]),
                        ap(p8, zsrc + 20 + q0,
                           [[20, 3], [zstp - zsrc, 2], [1, w]]),
                    )
            eng.tensor_scalar_mul(
                ap(f123, 320 + q0, [[480, 3], [20, 8], [1, w]]),
                ap(f123, 160 + q0, [[480, 3], [20, 8], [1, w]]),
                -1.0,
            )

        def emit_c01(q0, w, eng):
            # 8 mults per (m0, k, t), nesting (m1, a1, a0, q)
            # t=0 (re): k0 = F0re*F1re, k1 = F0im*F1nim
            # t=1 (im): k0 = F0re*F1im, k1 = F0im*F1re
            F1P = {(0, 0): 0, (0, 1): 320, (1, 0): 160, (1, 1): 0}
            for m0 in (0, 1):
                for t in (0, 1):
                    for kpl, fp in ((0, "re"), (1, "im")):
                        b0, s0 = F0B[(fp, m0)]
                        eng.tensor_tensor(
                            ap(cab, t * 640 + kpl * 320 + m0 * 40 + q0,
                               [[20, 2], [80, 4], [1, w]]),
                            ap(p8, b0 + q0, [[0, 4], [s0, 2], [1, w]]),
                            ap(f123, F1P[(t, kpl)] + q0, [[20, 8], [1, w]]),
                            MUL,
                        )
            eng.tensor_add(
                ap(c01, q0, [[320, 2], [20, 16], [1, w]]),
                ap(cab, q0, [[640, 2], [20, 16], [1, w]]),
                ap(cab, 320 + q0, [[640, 2], [20, 16], [1, w]]),
            )
            eng.tensor_add(
                ap(c01, 640 + q0, [[20, 16], [1, w]]),
                ap(c01, q0, [[20, 16], [1, w]]),
                ap(c01, 320 + q0, [[20, 16], [1, w]]),
            )

        def emit_c23(q0, w, eng):
            # 8 mults per (kpair, k, m2), nesting (m3, a3, a2, a1, q)
            F2, F3 = 480, 960
            F3P = ((0, 320), (160, 0))
            for kpair in (0, 1):
                for k in (0, 1):
                    for m2 in (0, 1):
                        eng.tensor_tensor(
                            ap(ccd, kpair * 1280 + k * 640 + m2 * 320 + q0,
                               [[20, 16], [1, w]]),
                            ap(f123, F2 + k * 160 + m2 * 80 + q0,
                               [[0, 4], [20, 4], [1, w]]),
                            ap(f123, F3 + F3P[kpair][k] + q0,
                               [[20, 8], [0, 2], [1, w]]),
                            MUL,
                        )
            # folded add: re @1280, imt @1920
            eng.tensor_add(
                ap(c23, 1280 + q0, [[640, 2], [20, 32], [1, w]]),
                ap(ccd, q0, [[1280, 2], [20, 32], [1, w]]),
                ap(ccd, 640 + q0, [[1280, 2], [20, 32], [1, w]]),
            )
            d32 = [[20, 32], [1, w]]
            eng.tensor_sub(ap(c23, q0, d32), ap(c23, 1280 + q0, d32),
                           ap(c23, 1920 + q0, d32))
            eng.tensor_add(ap(c23, 640 + q0, d32), ap(c23, 1280 + q0, d32),
                           ap(c23, 1920 + q0, d32))

        def emit_chunk(ci, eng_m):
            qb, ns = CHUNKS[ci]
            o = och[ci]
            ts = t_e if ci % 2 == 0 else t_o
            K = 256 * ns
            # 6 mults per (k, a1): nesting (a0, m01, m23, a3, a2, site);
            # T out is contiguous, c01 merges to [[20,8],[0,16],[1,ns]],
            # c23 to [[0,8],[40,16],[1,ns]]
            for k in (0, 1, 2):
                for a1 in (0, 1):
                    eng_m.tensor_tensor(
                        ap(ts, k * K + a1 * 128 * ns,
                           [[64 * ns, 2], [16 * ns, 4], [4 * ns, 4],
                            [2 * ns, 2], [ns, 2], [1, ns]]),
                        ap(c01, k * 320 + a1 * 160 + qb,
                           [[80, 2], [20, 4], [0, 16], [1, ns]]),
                        ap(c23, k * 640 + a1 * 20 + qb,
                           [[0, 8], [160, 4], [80, 2], [40, 2], [1, ns]]),
                        MUL,
                    )
            # 8 subs per (a1, a0, re/im): nesting (site, m01, m23, a3, a2);
            # the T side chain-merges to [[1,ns],[ns,64]]
            for a1 in (0, 1):
                for a0 in (0, 1):
                    eng_s = (nc.vector if SUBENG[ci][a1 * 2 + a0] == "v"
                             else nc.gpsimd)
                    base = a1 * 128 * ns + a0 * 64 * ns
                    ob = a1 * 16 + a0 * 32
                    sdim = [[1, ns], [16 * ns, 4], [4 * ns, 4], [2 * ns, 2], [ns, 2]]
                    odim = [[1024, ns], [256, 4], [64, 4], [6, 2], [8, 2]]
                    eng_s.tensor_sub(
                        ap(o, ob, odim),
                        ap(ts, 2 * K + base, sdim),
                        ap(ts, K + base, sdim),
                    )
                    eng_s.tensor_sub(
                        ap(o, ob + 1, odim),
                        ap(ts, 2 * K + base, sdim),
                        ap(ts, base, sdim),
                    )

        def emit_holes(ci):
            qb, ns = CHUNKS[ci]
            nc.scalar.copy(
                ap(och[ci], 2, [[8, 128 * ns], [1, 4]]),
                ap(zt, 0, [[0, 128 * ns], [1, 4]]),
            )

        # ======== Pool: slice-2 prologue (single slice q in [8,20)) ========
        emit_p8(8, 12, nc.gpsimd)
        emit_tables(8, 12, nc.gpsimd)

        # ======== DVE: slice 1 (q in [0,8)) ================================
        emit_p8(0, 8, nc.vector)
        emit_tables(0, 8, nc.vector)
        emit_c01(0, 8, nc.vector)
        emit_c23(0, 8, nc.vector)

        emit_holes(0)
        emit_chunk(0, nc.vector)
        nc.sync.dma_start(out_d[:, 1024:3072], och[0][:])


        for ci in (1, 2):
            with tc.tile_wait_until(ci):
                emit_holes(ci)
                emit_chunk(ci, nc.vector)
                qb, ns = CHUNKS[ci]
                nc.sync.dma_start(out_d[:, qb * 1024 : (qb + ns) * 1024], och[ci][:])

        # ======== Pool: slice-2 c01/c23 ====================================
        emit_c01(8, 12, nc.gpsimd)
        emit_c23(8, 12, nc.gpsimd)

        # ======== DVE: chunk D =============================================
        with tc.tile_wait_until(3):
            emit_holes(3)
            emit_chunk(3, nc.vector)
            _qb, _ns = CHUNKS[3]
            nc.sync.dma_start(out_d[:, _qb * 1024 : (_qb + _ns) * 1024], och[3][:])
        with tc.tile_wait_until(4):
            emit_holes(4)
            emit_chunk(4, nc.vector)
            _qb, _ns = CHUNKS[4]
            nc.sync.dma_start(out_d[:, _qb * 1024 : (_qb + _ns) * 1024], och[4][:])
        tc.tile_set_cur_wait(5)
        # site 19 (DVE): one mult per k (contiguous t19 block), then reduce
        # a0 (stride 64), a1 (stride 128->64), a2 (stride 1), then combine
        for k in (0, 1, 2):
            nc.vector.tensor_tensor(
                ap(t19, k * 256,
                   [[128, 2], [64, 2], [16, 4], [4, 4], [2, 2], [1, 2]]),
                ap(c01, k * 320 + 19, [[160, 2], [80, 2], [20, 4], [0, 16]]),
                ap(c23, k * 640 + 19,
                   [[20, 2], [0, 8], [160, 4], [80, 2], [40, 2]]),
                MUL,
            )
        nc.vector.tensor_add(
            ap(r1, 0, [[128, 3], [1, 128]]),
            ap(t19, 0, [[256, 3], [128, 2], [1, 64]]),
            ap(t19, 64, [[256, 3], [128, 2], [1, 64]]),
        )
        nc.vector.tensor_add(
            ap(r2, 0, [[64, 3], [1, 64]]),
            ap(r1, 0, [[128, 3], [1, 64]]),
            ap(r1, 64, [[128, 3], [1, 64]]),
        )
        nc.vector.tensor_add(
            ap(r3, 0, [[32, 3], [1, 32]]),
            ap(r2, 0, [[64, 3], [2, 32]]),
            ap(r2, 1, [[64, 3], [2, 32]]),
        )
        nc.vector.tensor_sub(
            ap(s19f, 0, [[4, 16], [2, 2], [1, 2]]),
            ap(r3, 64, [[2, 16], [1, 2], [0, 2]]),
            ap(r3, 32, [[2, 16], [1, 2], [-32, 2]]),
        )
        nc.scalar.copy(outd[:], s19f[:])

        for k in (0, 1, 2):
            for a1 in (0, 1):
                nc.vector.tensor_tensor(
                    ap(t0s, k * 16 + a1 * 4, [[8, 2], [2, 2], [1, 2]]),
                    ap(c01, k * 320 + a1 * 160, [[80, 2], [0, 4]]),
                    ap(c23, k * 640 + a1 * 20, [[0, 2], [40, 2], [80, 2]]),
                    MUL,
                )
        nc.vector.tensor_sub(
            ap(out0, 0, [[32, 2], [16, 2], [8, 2], [6, 2], [1, 2]]),
            ap(t0s, 32, [[8, 2], [4, 2], [2, 2], [1, 2], [0, 2]]),
            ap(t0s, 16, [[8, 2], [4, 2], [2, 2], [1, 2], [-16, 2]]),
        )
        nc.scalar.copy(ap(out0, 2, [[8, 8], [1, 4]]), ap(zt, 0, [[0, 8], [1, 4]]))

        nc.sync.dma_start(
            dram(19 * 1024, [[64, 16], [1, 4]]), ap(outd, 0, [[4, 16], [1, 4]])
        )
        nc.sync.dma_start(dram(0, [[1, 64]]), out0[:])

        with tc.tile_wait_until(6):
            emit_holes(5)
            emit_chunk(5, nc.vector)
            qb, ns = CHUNKS[5]
            nc.sync.dma_start(out_d[:, qb * 1024 : (qb + ns) * 1024], och[5][:])

    nc.compile()
    return nc


def _get_nc():
    if "nc" not in _CACHE:
        _CACHE["nc"] = _build_nc()
    return _CACHE["nc"]


def kernel(theta, batch_size):
    from concourse.bass_utils import run_bass_kernel_spmd

    theta = np.ascontiguousarray(np.asarray(theta), dtype=np.float32)
    assert theta.shape == (B_TOTAL, P_COLS)
    nc = _get_nc()
    in_maps = [{"theta": theta[c * B : (c + 1) * B]} for c in range(N_CORES)]
    res = run_bass_kernel_spmd(nc, in_maps, core_ids=list(range(N_CORES)))
    _CACHE["last_res"] = res
    full = np.concatenate([r["out"] for r in res.results], axis=0)
    return full.view(np.complex64).reshape(B_TOTAL, NQ, 16, 16, 2)
